# revision 3
# baseline (speedup 1.0000x reference)
"""Trainium2 Bass kernel: GPT2-style windowed attention (DecisionTransformer).

Full-input contract: kernel(**inputs) -> [B, S, D] float32.

Sharding: batch*heads across 8 cores (core c -> batch c//4, heads 4*(c%4)..+4).
Each core: column-sliced c_attn, full windowed attention for its 4 heads,
row-sliced c_proj producing a partial [S, D] output; host sums partials
(the "all-reduce") and adds c_proj bias + V-bias contribution once.

Layout / schedule choices:
  - all matmul operands are bf16 (tolerance is 2e-2; bf16 keeps PE at
    1 cyc/row and halves HBM traffic). PSUM accumulation stays fp32.
  - hidden is sent pre-transposed (xT [D, S]) so QK projections emit
    qT/kT directly in [head*dim, seq] layout; V is projected in
    [seq, head*dim] layout with a ones-column per head so attn@V
    accumulates softmax denominators in PSUM row 64 for free. V-bias
    folds into the host-side reduce (softmax rows sum to 1).
  - the whole kernel is software-pipelined: projections advance per
    512-col seq chunk, and attention q-quarters (scores+exp for all 4
    heads, kj-major attn@V, normalize, c_proj slice, output DMA) are
    issued as soon as their chunk dependencies are met. ACT does only
    exp during attention (the gating engine); evacuations go to DVE.
  - rope: rotate_half via 4 cross-quadrant 32-partition copies (2 DVE,
    2 GPSIMD) -- no DMA, no partition-swap latency chain.
  - scores for a head pair (partitions 0-63 / 64-127, K=64 each) are
    issued back-to-back so the PE runs them as concurrent row-tiles.
  - a dozen dummy matmuls at t=0 keep the PE busy (and the HAM
    clock-gate open) while the first input DMAs land.
"""

import sys

import numpy as np

sys.path.insert(0, "/opt/trn_rl_repo")

B, S, D = 2, 2048, 1024
H, HD = 16, 64
WINDOW = 512
ROPE_BASE = 4000.0
NCORES = 8
NH = 4          # heads per core
KT = D // 128   # 8 contraction tiles for c_attn
NB = S // 128   # 16 seq blocks
WB = WINDOW // 128  # 4 -> band spans up to 5 query blocks per key block


def _build_nc(debug_taps=False):
    import concourse.bass as bass
    from concourse import bacc, library_config, mybir
    import concourse.tile as tile

    f32 = mybir.dt.float32
    bf16 = mybir.dt.bfloat16
    Exp = mybir.ActivationFunctionType.Exp
    mult = mybir.AluOpType.mult
    ts = bass.ts
    ds = bass.ds

    nc = bacc.Bacc("TRN2")

    xT_d = nc.dram_tensor("xT", [D, S], bf16, kind="ExternalInput")
    wqkv_d = nc.dram_tensor("wqkv", [D, 3 * NH * HD], bf16, kind="ExternalInput")
    bqk_d = nc.dram_tensor("bqk", [128, 4], f32, kind="ExternalInput")
    wp_d = nc.dram_tensor("wp", [NH * HD, D], bf16, kind="ExternalInput")
    cos2_d = nc.dram_tensor("cos2", [128, S], bf16, kind="ExternalInput")
    sin2_d = nc.dram_tensor("sin2", [128, S], bf16, kind="ExternalInput")
    m0_d = nc.dram_tensor("m0", [128, 128], bf16, kind="ExternalInput")
    m4_d = nc.dram_tensor("m4", [128, 128], bf16, kind="ExternalInput")
    out_d = nc.dram_tensor("out", [S, D], bf16, kind="ExternalOutput")

    HS = S // 4  # q-quarter span: po is 1 PSUM bank
    QB = NB // 4  # 4 q-blocks per quarter

    with tile.TileContext(nc) as tc:
        nc.gpsimd.load_library(library_config.attn)

        with (
            tc.tile_pool(name="persist", bufs=1) as pp,
            tc.tile_pool(name="ps", bufs=2, space="PSUM") as ps_pool,
            tc.tile_pool(name="pso", bufs=3, space="PSUM") as pso_pool,
            tc.tile_pool(name="xw", bufs=1) as xw_pool,
            tc.tile_pool(name="ropetmp", bufs=2) as tmp_pool,
            tc.tile_pool(name="et", bufs=36) as e_pool,
            tc.tile_pool(name="rb", bufs=3) as rb_pool,
            tc.tile_pool(name="yo", bufs=3) as y_pool,
        ):
            # prewarm tile: zeros, matmul'd while input DMAs land
            zb = pp.tile([128, 512], bf16, tag="zb")
            nc.vector.memset(zb[:], 0.0)

            bqk_t = pp.tile([128, 4], f32, tag="bqk")
            nc.sync.dma_start(bqk_t[:], bqk_d[:])
            m0t = pp.tile([128, 128], bf16, tag="m0")
            nc.sync.dma_start(m0t[:], m0_d[:])
            m4t = pp.tile([128, 128], bf16, tag="m4")
            nc.sync.dma_start(m4t[:], m4_d[:])

            qk = [
                pp.tile([128, S], bf16, tag=f"qk{c}", name=f"qk{c}") for c in range(4)
            ]
            CV = NH * 65  # 260: per head 64 v-cols + 1 ones col
            vbig = pp.tile([128, NB, CV], bf16, tag="vbig")
            outH = pp.tile([128, 2, S], bf16, tag="outH")
            wpt = pp.tile([128, 2, D], bf16, tag="wpt")

            wbig = xw_pool.tile([128, KT, 3 * NH * HD], bf16, tag="wbig")
            xbig = xw_pool.tile([128, KT, S], bf16, tag="xbig")
            cos2 = xw_pool.tile([128, S], bf16, tag="cos2")
            sin2 = xw_pool.tile([128, S], bf16, tag="sin2")

            # DMA order = need order: v-cols, x cols 0:1024, qk-cols,
            # rope tables, x cols 1024:2048, c_proj weights. x moves in
            # 1024-col slices (2 KB contiguous lines) for DMA efficiency.
            VC = 2 * NH * HD
            for kt in range(KT):
                nc.sync.dma_start(wbig[:, kt, VC:], wqkv_d[ts(kt, 128), VC:])
            for kt in range(KT):
                nc.sync.dma_start(xbig[:, kt, 0:1024], xT_d[ts(kt, 128), 0:1024])
            for kt in range(KT):
                nc.sync.dma_start(wbig[:, kt, 0:VC], wqkv_d[ts(kt, 128), 0:VC])
            nc.sync.dma_start(cos2[:], cos2_d[:])
            nc.sync.dma_start(sin2[:], sin2_d[:])
            for kt in range(KT):
                nc.sync.dma_start(
                    xbig[:, kt, 1024:2048], xT_d[ts(kt, 128), 1024:2048]
                )
            for k2 in range(2):
                nc.sync.dma_start(wpt[:, k2, :], wp_d[ts(k2, 128), :])

            # PE prewarm: keep the tensor engine busy (and the HAM
            # clock-gate open) while the first input DMAs land.
            for w in range(12):
                psw = ps_pool.tile([128, 768], f32, tag="ps", name="psw")
                nc.tensor.matmul(
                    psw[:, 0:512], zb[:, 0:128], zb[:, 0:512],
                    start=True, stop=True,
                )

            # ---------------- building blocks ----------------
            eTs = [dict() for _ in range(NH)]  # [h][kj] -> masked exp'd scores

            def proj_chunk(sc):
                # V projection for this chunk's 4 seq blocks
                for sb in range(4 * sc, 4 * sc + 4):
                    vsb = vbig[:, sb, :].rearrange("p (h c) -> p h c", c=65)
                    nc.vector.memset(vsb[:, :, 64:65], 1.0)
                    psv = ps_pool.tile([128, 768], f32, tag="ps", name="psv")
                    for kt in range(KT):
                        nc.tensor.matmul(
                            psv[:, 0 : NH * HD],
                            xbig[:, kt, ts(sb, 128)],
                            wbig[:, kt, ds(2 * NH * HD, NH * HD)],
                            start=(kt == 0),
                            stop=(kt == KT - 1),
                        )
                    nc.vector.tensor_copy(
                        vsb[:, :, 0:64],
                        psv[:, 0 : NH * HD].rearrange("p (h c) -> p h c", c=64),
                    )
                # Q/K projections + rope; head pair 0's tiles (c=0,2) first
                for c in (0, 2, 1, 3):
                    psb = ps_pool.tile([128, 768], f32, tag="ps", name="psb")
                    for kt in range(KT):
                        nc.tensor.matmul(
                            psb[:, 0:512],
                            wbig[:, kt, ts(c, 128)],
                            xbig[:, kt, ts(sc, 512)],
                            start=(kt == 0),
                            stop=(kt == KT - 1),
                        )
                    nc.scalar.add(
                        qk[c][:, ts(sc, 512)], psb[:, 0:512], bqk_t[:, c : c + 1]
                    )
                    # rope: rotate_half via cross-quadrant 32-partition
                    # copies (sign is folded into the sin table)
                    qc = qk[c][:, ts(sc, 512)]
                    tmp = tmp_pool.tile([128, 512], bf16, tag="ropetmp", name="tmp")
                    nc.vector.tensor_copy(tmp[0:32, :], qk[c][32:64, ts(sc, 512)])
                    nc.gpsimd.tensor_copy(tmp[32:64, :], qk[c][0:32, ts(sc, 512)])
                    nc.vector.tensor_copy(tmp[64:96, :], qk[c][96:128, ts(sc, 512)])
                    nc.gpsimd.tensor_copy(tmp[96:128, :], qk[c][64:96, ts(sc, 512)])
                    nc.vector.tensor_tensor(
                        tmp[:], tmp[:], sin2[:, ts(sc, 512)], op=mult
                    )
                    nc.vector.tensor_tensor(qc, qc, cos2[:, ts(sc, 512)], op=mult)
                    nc.vector.tensor_add(qc, qc, tmp[:])

            def scores_exp(h, kj):
                # transposed scores sT[k, q] for the full band of kj
                # (5 q-blocks), exp'd on ACT, masked on DVE
                hb = (h % 2) * 64
                qt = qk[h // 2]
                kt_ = qk[2 + h // 2]
                nq = min(WB + 1, NB - kj)
                pss = ps_pool.tile([128, 768], f32, tag="ps", name="pss")
                n1 = min(512, nq * 128)
                n2 = nq * 128 - n1
                lhs_k = kt_[hb : hb + 64, ts(kj, 128)]
                nc.tensor.matmul(
                    pss[:, 0:n1],
                    lhs_k,
                    qt[hb : hb + 64, ds(kj * 128, n1)],
                    start=True,
                    stop=True,
                )
                if n2:
                    nc.tensor.matmul(
                        pss[:, 512 : 512 + n2],
                        lhs_k,
                        qt[hb : hb + 64, ds(kj * 128 + 512, n2)],
                        start=True,
                        stop=True,
                    )
                return pss, nq

            def exp_mask(h, kj, pss, nq):
                eT = e_pool.tile([128, 640], bf16, tag="et", name="eT")
                nc.scalar.activation(
                    eT[:, 0 : nq * 128], pss[:, 0 : nq * 128], Exp, scale=0.125
                )
                # banded mask: diag block keeps kk<=qq, window edge kk>qq
                nc.vector.tensor_tensor(eT[:, 0:128], eT[:, 0:128], m0t[:], op=mult)
                if nq == WB + 1:
                    nc.vector.tensor_tensor(
                        eT[:, 512:640], eT[:, 512:640], m4t[:], op=mult
                    )
                eTs[h][kj] = eT

            def evac_q(po, h, qtr):
                # normalize by denominators (PSUM row 64) into outH
                hb = (h % 2) * 64
                rb = rb_pool.tile([64, HS], f32, tag="rb", name="rb")
                nc.vector.tensor_copy(rb[0:1, :], po[64:65, :])
                nc.vector.reciprocal_approx_fast(rb[0:1, :], rb[0:1, :])
                nc.gpsimd.partition_broadcast(rb[:], rb[0:1, :])
                nc.vector.tensor_tensor(
                    outH[hb : hb + 64, h // 2, qtr * HS : (qtr + 1) * HS],
                    po[0:64, :],
                    rb[:],
                    op=mult,
                )

            def quarter_block(qtr):
                # fresh scores for kj = 4q..4q+3, pair-interleaved so the
                # two heads' K=64 matmuls run as concurrent PE row-tiles
                for kj in range(4 * qtr, 4 * qtr + 4):
                    for pair in range(2):
                        s0 = scores_exp(2 * pair, kj)
                        s1 = scores_exp(2 * pair + 1, kj)
                        exp_mask(2 * pair, kj, *s0)
                        exp_mask(2 * pair + 1, kj, *s1)
                # kj-major attn@V per head: each V block loads once and
                # streams its whole q-span (per-element has_written bits
                # handle the staggered accumulation regions)
                kjlo = max(0, 4 * qtr - WB)
                kjhi = 4 * qtr + QB - 1
                for h in range(NH):
                    po = pso_pool.tile([65, HS], f32, tag="pso", name="po")
                    for kj in range(kjlo, kjhi + 1):
                        qlo = max(4 * qtr, kj)
                        qhi = min(4 * qtr + QB - 1, kj + WB)
                        off = (qlo - kj) * 128
                        n = (qhi - qlo + 1) * 128
                        nc.tensor.matmul(
                            po[:, ds((qlo - 4 * qtr) * 128, n)],
                            vbig[:, kj, h * 65 : h * 65 + 65],
                            eTs[h][kj][:, off : off + n],
                            start=(kj == kjlo),
                            stop=(kj == kjhi),
                        )
                    evac_q(po, h, qtr)
                # c_proj for this quarter's 4 seq blocks + output DMA
                for sb in range(4 * qtr, 4 * qtr + 4):
                    psp = ps_pool.tile([128, 1024], f32, tag="ps", name="psp")
                    for k2 in range(2):
                        for ncol in range(2):
                            nc.tensor.matmul(
                                psp[:, ts(ncol, 512)],
                                outH[:, k2, ts(sb, 128)],
                                wpt[:, k2, ts(ncol, 512)],
                                start=(k2 == 0),
                                stop=(k2 == 1),
                            )
                    yt = y_pool.tile([128, D], bf16, tag="yo", name="yt")
                    if sb % 2 == 0:
                        nc.scalar.copy(yt[:], psp[:])
                    else:
                        nc.vector.tensor_copy(yt[:], psp[:])
                    nc.sync.dma_start(out_d[ts(sb, 128), :], yt[:])

            # ---------------- pipelined schedule ----------------
            proj_chunk(0)
            proj_chunk(1)
            quarter_block(0)
            proj_chunk(2)
            quarter_block(1)
            proj_chunk(3)
            quarter_block(2)
            quarter_block(3)

    nc.compile()
    return nc


def _host_inputs(hidden, pos, caw, cab, cpw):
    """Build the 8 per-core input maps."""
    inv = 1.0 / (ROPE_BASE ** (np.arange(0, HD, 2, dtype=np.float32) / HD))
    t = np.arange(S, dtype=np.float32)
    freqs = np.outer(t, inv).astype(np.float32)
    emb = np.concatenate([freqs, freqs], axis=1)  # [S, HD]
    cos = np.cos(emb).astype(np.float32)
    sin = np.sin(emb).astype(np.float32)

    import ml_dtypes

    bf = ml_dtypes.bfloat16
    ii = np.arange(128)
    m0 = (ii[:, None] <= ii[None, :]).astype(bf)
    m4 = (ii[:, None] > ii[None, :]).astype(bf)

    xTs, cos2s, sin2s = [], [], []
    for b in range(B):
        xTs.append(np.ascontiguousarray(hidden[b].T).astype(bf))
        cosT = np.ascontiguousarray(cos[pos[b]].T)  # [HD, S]
        sinT = np.ascontiguousarray(sin[pos[b]].T)
        sinS = np.concatenate([-sinT[:32], sinT[32:]], axis=0)
        cos2s.append(np.tile(cosT, (2, 1)).astype(bf))
        sin2s.append(np.tile(sinS, (2, 1)).astype(bf))

    in_maps = []
    for c in range(NCORES):
        b = c // 4
        h0 = NH * (c % 4)
        col = h0 * HD
        w_q = caw[:, col : col + NH * HD]
        w_k = caw[:, D + col : D + col + NH * HD]
        w_v = caw[:, 2 * D + col : 2 * D + col + NH * HD]
        wqkv = np.ascontiguousarray(
            np.concatenate([w_q, w_k, w_v], axis=1)
        ).astype(bf)
        b_q = cab[col : col + NH * HD]
        b_k = cab[D + col : D + col + NH * HD]
        bqk = np.ascontiguousarray(
            np.concatenate([b_q, b_k]).reshape(4, 128).T
        )  # [128, 4]: partition = col within tile
        wp = np.ascontiguousarray(cpw[col : col + NH * HD, :]).astype(bf)
        in_maps.append(
            {
                "xT": xTs[b],
                "wqkv": wqkv,
                "bqk": bqk,
                "wp": wp,
                "cos2": cos2s[b],
                "sin2": sin2s[b],
                "m0": m0,
                "m4": m4,
            }
        )
    return in_maps


def _assemble(results, cab, cpw, cpb):
    """Host all-reduce of the 4 per-batch partials + biases.

    The V-bias contribution is position-independent after softmax
    (attn rows sum to 1), so it folds into a constant row vector:
    bias_v @ c_proj_w.
    """
    vrow = cab[2 * D :].astype(np.float32) @ cpw.astype(np.float32)
    bias = cpb.astype(np.float32) + vrow
    y = np.empty((B, S, D), dtype=np.float32)
    for b in range(B):
        acc = results[4 * b]["out"].astype(np.float32)
        for c in range(4 * b + 1, 4 * b + 4):
            acc = acc + results[c]["out"].astype(np.float32)
        y[b] = acc + bias[None, :]
    return y


def kernel(**inputs):
    from concourse import bass_utils

    hidden = np.asarray(inputs["hidden_states"], dtype=np.float32)
    pos = np.asarray(inputs["position_ids"]).astype(np.int64)
    caw = np.asarray(inputs["c_attn_w"], dtype=np.float32)
    cab = np.asarray(inputs["c_attn_b"], dtype=np.float32)
    cpw = np.asarray(inputs["c_proj_w"], dtype=np.float32)
    cpb = np.asarray(inputs["c_proj_b"], dtype=np.float32)

    in_maps = _host_inputs(hidden, pos, caw, cab, cpw)
    nc = _build_nc()
    res = bass_utils.run_bass_kernel_spmd(nc, in_maps, list(range(NCORES)))
    return _assemble(res.results, cab, cpw, cpb)


# revision 10
# speedup vs baseline: 1.4825x; 1.4825x over previous
"""Trainium2 Bass kernel: GPT2-style windowed attention (DecisionTransformer).

Full-input contract: kernel(**inputs) -> [B, S, D] float32.

Sharding: batch*heads across 8 cores (core c -> batch c//4, heads 4*(c%4)..+4).
Each core: column-sliced c_attn, full windowed attention for its 4 heads,
row-sliced c_proj producing a partial [S, D] output; host sums partials
(the "all-reduce") and adds c_proj bias + V-bias contribution once.

Layout / schedule choices:
  - all matmul operands are bf16 (tolerance is 2e-2; bf16 keeps PE at
    1 cyc/row and halves HBM traffic). PSUM accumulation stays fp32.
  - hidden is sent pre-transposed (xT [D, S]) so QK projections emit
    qT/kT directly in [head*dim, seq] layout; V is projected in
    [seq, head*dim] layout with a ones-column per head so attn@V
    accumulates softmax denominators in PSUM row 64 for free. V-bias
    folds into the host-side reduce (softmax rows sum to 1).
  - the whole kernel is software-pipelined: projections advance per
    512-col seq chunk, and attention q-quarters (scores+exp for all 4
    heads, kj-major attn@V, normalize, c_proj slice, output DMA) are
    issued as soon as their chunk dependencies are met. ACT does only
    exp during attention (the gating engine); evacuations go to DVE.
  - rope: rotate_half via 4 cross-quadrant 32-partition copies (2 DVE,
    2 GPSIMD) -- no DMA, no partition-swap latency chain.
  - scores for a head pair (partitions 0-63 / 64-127, K=64 each) are
    issued back-to-back so the PE runs them as concurrent row-tiles.
  - a dozen dummy matmuls at t=0 keep the PE busy (and the HAM
    clock-gate open) while the first input DMAs land.
"""

import sys

import numpy as np

sys.path.insert(0, "/opt/trn_rl_repo")

B, S, D = 2, 2048, 1024
H, HD = 16, 64
WINDOW = 512
ROPE_BASE = 4000.0
NCORES = 8
NH = 4          # heads per core
KT = D // 128   # 8 contraction tiles for c_attn
NB = S // 128   # 16 seq blocks
WB = WINDOW // 128  # 4 -> band spans up to 5 query blocks per key block


def _build_nc(debug_taps=False):
    import concourse.bass as bass
    from concourse import bacc, library_config, mybir
    import concourse.tile as tile

    f32 = mybir.dt.float32
    bf16 = mybir.dt.bfloat16
    Exp = mybir.ActivationFunctionType.Exp
    mult = mybir.AluOpType.mult
    ts = bass.ts
    ds = bass.ds

    nc = bacc.Bacc("TRN2")

    xT_d = nc.dram_tensor("xT", [D, S], bf16, kind="ExternalInput")
    wqkv_d = nc.dram_tensor("wqkv", [D, 3 * NH * HD], bf16, kind="ExternalInput")
    bqk_d = nc.dram_tensor("bqk", [128, 4], f32, kind="ExternalInput")
    wp_d = nc.dram_tensor("wp", [NH * HD, D], bf16, kind="ExternalInput")
    cos2_d = nc.dram_tensor("cos2", [128, S], bf16, kind="ExternalInput")
    sin2_d = nc.dram_tensor("sin2", [128, S], bf16, kind="ExternalInput")
    m04_d = nc.dram_tensor("m04", [128, 256], bf16, kind="ExternalInput")
    out_d = nc.dram_tensor("out", [S, D], bf16, kind="ExternalOutput")

    HS = S // 4  # q-quarter span: po is 1 PSUM bank
    QB = NB // 4  # 4 q-blocks per quarter

    with tile.TileContext(nc) as tc:
        nc.gpsimd.load_library(library_config.attn)

        with (
            tc.tile_pool(name="persist", bufs=1) as pp,
            tc.tile_pool(name="ps", bufs=2, space="PSUM") as ps_pool,
            tc.tile_pool(name="pso", bufs=3, space="PSUM") as pso_pool,
            tc.tile_pool(name="xw", bufs=1) as xw_pool,
            tc.tile_pool(name="ropetmp", bufs=2) as tmp_pool,
            tc.tile_pool(name="et", bufs=36) as e_pool,
            tc.tile_pool(name="rb", bufs=3) as rb_pool,
            tc.tile_pool(name="yo", bufs=3) as y_pool,
        ):
            # prewarm tile: zeros, matmul'd while input DMAs land
            zb = pp.tile([128, 512], bf16, tag="zb")
            nc.vector.memset(zb[:], 0.0)

            bqk_t = pp.tile([128, 4], f32, tag="bqk")
            nc.sync.dma_start(bqk_t[:], bqk_d[:])
            # m04[:, 0, :] = diag-block mask (kk<=qq); [:, 1, :] = window
            # edge mask (kk>qq) — applied as one two-region strided op
            m04t = pp.tile([128, 2, 128], bf16, tag="m04")
            nc.sync.dma_start(
                m04t[:], m04_d[:].rearrange("p (a b) -> p a b", a=2)
            )

            qk = [
                pp.tile([128, S], bf16, tag=f"qk{c}", name=f"qk{c}") for c in range(4)
            ]
            CV = NH * 65  # 260: per head 64 v-cols + 1 ones col
            vbig = pp.tile([128, NB, CV], bf16, tag="vbig")
            outH = pp.tile([128, 2, S], bf16, tag="outH")
            wpt = pp.tile([128, 2, D], bf16, tag="wpt")

            wbig = xw_pool.tile([128, KT, 3 * NH * HD], bf16, tag="wbig")
            xbig = xw_pool.tile([128, KT, S], bf16, tag="xbig")
            cos2 = xw_pool.tile([128, S], bf16, tag="cos2")
            sin2 = xw_pool.tile([128, S], bf16, tag="sin2")

            # DMA order = need order: v-cols, x cols 0:1024, qk-cols,
            # rope tables, x cols 1024:2048, c_proj weights. x moves in
            # 1024-col slices (2 KB contiguous lines) for DMA efficiency.
            VC = 2 * NH * HD
            for kt in range(KT):
                nc.sync.dma_start(wbig[:, kt, VC:], wqkv_d[ts(kt, 128), VC:])
            for kt in range(KT):
                nc.sync.dma_start(xbig[:, kt, 0:1024], xT_d[ts(kt, 128), 0:1024])
            for kt in range(KT):
                nc.sync.dma_start(wbig[:, kt, 0:VC], wqkv_d[ts(kt, 128), 0:VC])
            nc.sync.dma_start(cos2[:], cos2_d[:])
            nc.sync.dma_start(sin2[:], sin2_d[:])
            for kt in range(KT):
                nc.sync.dma_start(
                    xbig[:, kt, 1024:2048], xT_d[ts(kt, 128), 1024:2048]
                )
            for k2 in range(2):
                nc.sync.dma_start(wpt[:, k2, :], wp_d[ts(k2, 128), :])

            # PE prewarm: keep the tensor engine busy (and the HAM
            # clock-gate open) while the first input DMAs land.
            for w in range(12):
                psw = ps_pool.tile([128, 768], f32, tag="ps", name="psw")
                nc.tensor.matmul(
                    psw[:, 0:512], zb[:, 0:128], zb[:, 0:512],
                    start=True, stop=True,
                )

            # ---------------- building blocks ----------------
            eTs = [dict() for _ in range(NH)]  # [h][kj] -> masked exp'd scores

            def proj_chunk(sc):
                # V projection for this chunk's 4 seq blocks
                for sb in range(4 * sc, 4 * sc + 4):
                    vsb = vbig[:, sb, :].rearrange("p (h c) -> p h c", c=65)
                    nc.vector.memset(vsb[:, :, 64:65], 1.0)
                    psv = ps_pool.tile([128, 768], f32, tag="ps", name="psv")
                    for kt in range(KT):
                        nc.tensor.matmul(
                            psv[:, 0 : NH * HD],
                            xbig[:, kt, ts(sb, 128)],
                            wbig[:, kt, ds(2 * NH * HD, NH * HD)],
                            start=(kt == 0),
                            stop=(kt == KT - 1),
                        )
                    nc.vector.tensor_copy(
                        vsb[:, :, 0:64],
                        psv[:, 0 : NH * HD].rearrange("p (h c) -> p h c", c=64),
                    )
                # Q/K projections + rope; head pair 0's tiles (c=0,2) first
                for c in (0, 2, 1, 3):
                    psb = ps_pool.tile([128, 768], f32, tag="ps", name="psb")
                    for kt in range(KT):
                        nc.tensor.matmul(
                            psb[:, 0:512],
                            wbig[:, kt, ts(c, 128)],
                            xbig[:, kt, ts(sc, 512)],
                            start=(kt == 0),
                            stop=(kt == KT - 1),
                        )
                    nc.scalar.add(
                        qk[c][:, ts(sc, 512)], psb[:, 0:512], bqk_t[:, c : c + 1]
                    )
                    # rope: rotate_half via partition-swap SBUF DMAs on the
                    # sync/gpsimd queues (sign is folded into the sin table)
                    qc = qk[c][:, ts(sc, 512)]
                    tmp = tmp_pool.tile([128, 512], bf16, tag="ropetmp", name="tmp")
                    dma_engs = [nc.sync, nc.gpsimd, nc.sync, nc.gpsimd]
                    for g in range(2):
                        b0 = g * 64
                        dma_engs[2 * g].dma_start(
                            tmp[b0 : b0 + 32, :],
                            qk[c][b0 + 32 : b0 + 64, ts(sc, 512)],
                        )
                        dma_engs[2 * g + 1].dma_start(
                            tmp[b0 + 32 : b0 + 64, :],
                            qk[c][b0 : b0 + 32, ts(sc, 512)],
                        )
                    nc.vector.tensor_tensor(
                        tmp[:], tmp[:], sin2[:, ts(sc, 512)], op=mult
                    )
                    nc.vector.tensor_tensor(qc, qc, cos2[:, ts(sc, 512)], op=mult)
                    nc.vector.tensor_add(qc, qc, tmp[:])

            def scores_exp(h, kj):
                # transposed scores sT[k, q] for the full band of kj
                # (5 q-blocks), exp'd on ACT, masked on DVE
                hb = (h % 2) * 64
                qt = qk[h // 2]
                kt_ = qk[2 + h // 2]
                nq = min(WB + 1, NB - kj)
                pss = ps_pool.tile([128, 768], f32, tag="ps", name="pss")
                n1 = min(512, nq * 128)
                n2 = nq * 128 - n1
                lhs_k = kt_[hb : hb + 64, ts(kj, 128)]
                nc.tensor.matmul(
                    pss[:, 0:n1],
                    lhs_k,
                    qt[hb : hb + 64, ds(kj * 128, n1)],
                    start=True,
                    stop=True,
                )
                if n2:
                    nc.tensor.matmul(
                        pss[:, 512 : 512 + n2],
                        lhs_k,
                        qt[hb : hb + 64, ds(kj * 128 + 512, n2)],
                        start=True,
                        stop=True,
                    )
                return pss, nq

            def exp_mask(h, kj, pss, nq):
                eT = e_pool.tile([128, 640], bf16, tag="et", name="eT")
                nc.scalar.activation(
                    eT[:, 0 : nq * 128], pss[:, 0 : nq * 128], Exp, scale=0.125
                )
                # banded mask: diag block keeps kk<=qq, window edge kk>qq.
                # Both 128-col regions are masked in one strided DVE op.
                if nq == WB + 1:
                    e2 = eT[:, 0:640].rearrange("p (x y) -> p x y", y=128)[
                        :, 0:5:4, :
                    ]
                    nc.vector.tensor_tensor(e2, e2, m04t[:], op=mult)
                else:
                    nc.vector.tensor_tensor(
                        eT[:, 0:128], eT[:, 0:128], m04t[:, 0, :], op=mult
                    )
                eTs[h][kj] = eT

            def evac_q(po, h, qtr):
                # normalize by denominators (PSUM row 64) into outH.
                # custom-DVE must not read PSUM: stage denom row via ACT.
                hb = (h % 2) * 64
                rb = rb_pool.tile([64, HS], f32, tag="rb", name="rb")
                nc.scalar.copy(rb[0:1, :], po[64:65, :])
                nc.vector.reciprocal_approx_fast(rb[0:1, :], rb[0:1, :])
                nc.gpsimd.partition_broadcast(rb[:], rb[0:1, :])
                nc.vector.tensor_tensor(
                    outH[hb : hb + 64, h // 2, qtr * HS : (qtr + 1) * HS],
                    po[0:64, :],
                    rb[:],
                    op=mult,
                )

            def quarter_block(qtr):
                # fresh scores for kj = 4q..4q+3, pair-interleaved so the
                # two heads' K=64 matmuls run as concurrent PE row-tiles
                for kj in range(4 * qtr, 4 * qtr + 4):
                    for pair in range(2):
                        s0 = scores_exp(2 * pair, kj)
                        s1 = scores_exp(2 * pair + 1, kj)
                        exp_mask(2 * pair, kj, *s0)
                        exp_mask(2 * pair + 1, kj, *s1)
                # kj-major attn@V per head: each V block loads once and
                # streams its whole q-span (per-element has_written bits
                # handle the staggered accumulation regions)
                kjlo = max(0, 4 * qtr - WB)
                kjhi = 4 * qtr + QB - 1
                for h in range(NH):
                    po = pso_pool.tile([65, HS], f32, tag="pso", name="po")
                    for kj in range(kjlo, kjhi + 1):
                        qlo = max(4 * qtr, kj)
                        qhi = min(4 * qtr + QB - 1, kj + WB)
                        off = (qlo - kj) * 128
                        n = (qhi - qlo + 1) * 128
                        nc.tensor.matmul(
                            po[:, ds((qlo - 4 * qtr) * 128, n)],
                            vbig[:, kj, h * 65 : h * 65 + 65],
                            eTs[h][kj][:, off : off + n],
                            start=(kj == kjlo),
                            stop=(kj == kjhi),
                        )
                    evac_q(po, h, qtr)
                # c_proj for this quarter's 4 seq blocks + output DMA
                for sb in range(4 * qtr, 4 * qtr + 4):
                    psp = ps_pool.tile([128, 1024], f32, tag="ps", name="psp")
                    for k2 in range(2):
                        for ncol in range(2):
                            nc.tensor.matmul(
                                psp[:, ts(ncol, 512)],
                                outH[:, k2, ts(sb, 128)],
                                wpt[:, k2, ts(ncol, 512)],
                                start=(k2 == 0),
                                stop=(k2 == 1),
                            )
                    yt = y_pool.tile([128, D], bf16, tag="yo", name="yt")
                    if sb % 2 == 0:
                        nc.scalar.copy(yt[:], psp[:])
                    else:
                        nc.vector.tensor_copy(yt[:], psp[:])
                    nc.sync.dma_start(out_d[ts(sb, 128), :], yt[:])

            # ---------------- pipelined schedule ----------------
            proj_chunk(0)
            proj_chunk(1)
            quarter_block(0)
            proj_chunk(2)
            quarter_block(1)
            proj_chunk(3)
            quarter_block(2)
            quarter_block(3)

    nc.compile()
    return nc


def _host_inputs(hidden, pos, caw, cab, cpw):
    """Build the 8 per-core input maps."""
    inv = 1.0 / (ROPE_BASE ** (np.arange(0, HD, 2, dtype=np.float32) / HD))
    t = np.arange(S, dtype=np.float32)
    freqs = np.outer(t, inv).astype(np.float32)
    emb = np.concatenate([freqs, freqs], axis=1)  # [S, HD]
    cos = np.cos(emb).astype(np.float32)
    sin = np.sin(emb).astype(np.float32)

    import ml_dtypes

    bf = ml_dtypes.bfloat16
    ii = np.arange(128)
    m0 = (ii[:, None] <= ii[None, :]).astype(bf)
    m4 = (ii[:, None] > ii[None, :]).astype(bf)
    m04 = np.ascontiguousarray(np.concatenate([m0, m4], axis=1))

    xTs, cos2s, sin2s = [], [], []
    for b in range(B):
        xTs.append(np.ascontiguousarray(hidden[b].T).astype(bf))
        cosT = np.ascontiguousarray(cos[pos[b]].T)  # [HD, S]
        sinT = np.ascontiguousarray(sin[pos[b]].T)
        sinS = np.concatenate([-sinT[:32], sinT[32:]], axis=0)
        cos2s.append(np.tile(cosT, (2, 1)).astype(bf))
        sin2s.append(np.tile(sinS, (2, 1)).astype(bf))

    in_maps = []
    for c in range(NCORES):
        b = c // 4
        h0 = NH * (c % 4)
        col = h0 * HD
        w_q = caw[:, col : col + NH * HD]
        w_k = caw[:, D + col : D + col + NH * HD]
        w_v = caw[:, 2 * D + col : 2 * D + col + NH * HD]
        wqkv = np.ascontiguousarray(
            np.concatenate([w_q, w_k, w_v], axis=1)
        ).astype(bf)
        b_q = cab[col : col + NH * HD]
        b_k = cab[D + col : D + col + NH * HD]
        bqk = np.ascontiguousarray(
            np.concatenate([b_q, b_k]).reshape(4, 128).T
        )  # [128, 4]: partition = col within tile
        wp = np.ascontiguousarray(cpw[col : col + NH * HD, :]).astype(bf)
        in_maps.append(
            {
                "xT": xTs[b],
                "wqkv": wqkv,
                "bqk": bqk,
                "wp": wp,
                "cos2": cos2s[b],
                "sin2": sin2s[b],
                "m04": m04,
            }
        )
    return in_maps


def _assemble(results, cab, cpw, cpb):
    """Host all-reduce of the 4 per-batch partials + biases.

    The V-bias contribution is position-independent after softmax
    (attn rows sum to 1), so it folds into a constant row vector:
    bias_v @ c_proj_w.
    """
    vrow = cab[2 * D :].astype(np.float32) @ cpw.astype(np.float32)
    bias = cpb.astype(np.float32) + vrow
    y = np.empty((B, S, D), dtype=np.float32)
    for b in range(B):
        acc = results[4 * b]["out"].astype(np.float32)
        for c in range(4 * b + 1, 4 * b + 4):
            acc = acc + results[c]["out"].astype(np.float32)
        y[b] = acc + bias[None, :]
    return y


def kernel(**inputs):
    from concourse import bass_utils

    hidden = np.asarray(inputs["hidden_states"], dtype=np.float32)
    pos = np.asarray(inputs["position_ids"]).astype(np.int64)
    caw = np.asarray(inputs["c_attn_w"], dtype=np.float32)
    cab = np.asarray(inputs["c_attn_b"], dtype=np.float32)
    cpw = np.asarray(inputs["c_proj_w"], dtype=np.float32)
    cpb = np.asarray(inputs["c_proj_b"], dtype=np.float32)

    in_maps = _host_inputs(hidden, pos, caw, cab, cpw)
    nc = _build_nc()
    res = bass_utils.run_bass_kernel_spmd(nc, in_maps, list(range(NCORES)))
    return _assemble(res.results, cab, cpw, cpb)


# revision 15
# speedup vs baseline: 1.5625x; 1.0539x over previous
"""Trainium2 Bass kernel: GPT2-style windowed attention (DecisionTransformer).

Full-input contract: kernel(**inputs) -> [B, S, D] float32.

Sharding: batch*heads across 8 cores (core c -> batch c//4, heads 4*(c%4)..+4).
Each core: column-sliced c_attn, full windowed attention for its 4 heads,
row-sliced c_proj producing a partial [S, D] output; host sums partials
(the "all-reduce") and adds c_proj bias + V-bias contribution once.

Layout / schedule choices:
  - all matmul operands are bf16 (tolerance is 2e-2; bf16 keeps PE at
    1 cyc/row and halves HBM traffic). PSUM accumulation stays fp32.
  - hidden is sent pre-transposed (xT [D, S]) so QK projections emit
    qT/kT directly in [head*dim, seq] layout; V is projected in
    [seq, head*dim] layout with a ones-column per head so attn@V
    accumulates softmax denominators in PSUM row 64 for free. V-bias
    folds into the host-side reduce (softmax rows sum to 1).
  - the whole kernel is software-pipelined: projections advance per
    512-col seq chunk, and attention q-quarters (scores+exp for all 4
    heads, kj-major attn@V, normalize, c_proj slice, output DMA) are
    issued as soon as their chunk dependencies are met. ACT does only
    exp during attention (the gating engine); evacuations go to DVE.
  - rope: rotate_half via 4 cross-quadrant 32-partition copies (2 DVE,
    2 GPSIMD) -- no DMA, no partition-swap latency chain.
  - scores for a head pair (partitions 0-63 / 64-127, K=64 each) are
    issued back-to-back so the PE runs them as concurrent row-tiles.
  - a dozen dummy matmuls at t=0 keep the PE busy (and the HAM
    clock-gate open) while the first input DMAs land.
"""

import sys

import numpy as np

sys.path.insert(0, "/opt/trn_rl_repo")

B, S, D = 2, 2048, 1024
H, HD = 16, 64
WINDOW = 512
ROPE_BASE = 4000.0
NCORES = 8
NH = 4          # heads per core
KT = D // 128   # 8 contraction tiles for c_attn
NB = S // 128   # 16 seq blocks
WB = WINDOW // 128  # 4 -> band spans up to 5 query blocks per key block


def _build_nc(debug_taps=False):
    import concourse.bass as bass
    from concourse import bacc, library_config, mybir
    import concourse.tile as tile

    f32 = mybir.dt.float32
    bf16 = mybir.dt.bfloat16
    Exp = mybir.ActivationFunctionType.Exp
    mult = mybir.AluOpType.mult
    ts = bass.ts
    ds = bass.ds

    nc = bacc.Bacc("TRN2")

    xT_d = nc.dram_tensor("xT", [D, S], bf16, kind="ExternalInput")
    wqkv_d = nc.dram_tensor("wqkv", [D, 3 * NH * HD], bf16, kind="ExternalInput")
    bqk_d = nc.dram_tensor("bqk", [128, 4], f32, kind="ExternalInput")
    wp_d = nc.dram_tensor("wp", [NH * HD, D], bf16, kind="ExternalInput")
    cos2_d = nc.dram_tensor("cos2", [128, S], bf16, kind="ExternalInput")
    sin2_d = nc.dram_tensor("sin2", [128, S], bf16, kind="ExternalInput")
    m04_d = nc.dram_tensor("m04", [128, 256], bf16, kind="ExternalInput")
    out_d = nc.dram_tensor("out", [S, D], bf16, kind="ExternalOutput")

    HS = S // 4  # q-quarter span: po is 1 PSUM bank
    QB = NB // 4  # 4 q-blocks per quarter

    with tile.TileContext(nc) as tc:
        nc.gpsimd.load_library(library_config.attn)

        with (
            tc.tile_pool(name="persist", bufs=1) as pp,
            tc.tile_pool(name="psj", bufs=2, space="PSUM") as psj_pool,
            tc.tile_pool(name="ps", bufs=2, space="PSUM") as ps_pool,
            tc.tile_pool(name="pso", bufs=2, space="PSUM") as pso_pool,
            tc.tile_pool(name="xw", bufs=1) as xw_pool,
            tc.tile_pool(name="ropetmp", bufs=2) as tmp_pool,
            tc.tile_pool(name="et", bufs=36) as e_pool,
            tc.tile_pool(name="rb", bufs=3) as rb_pool,
            tc.tile_pool(name="yo", bufs=3) as y_pool,
        ):
            # prewarm tile: zeros, matmul'd while input DMAs land
            zb = pp.tile([128, 512], bf16, tag="zb")
            nc.vector.memset(zb[:], 0.0)

            bqk_t = pp.tile([128, 4], f32, tag="bqk")
            nc.sync.dma_start(bqk_t[:], bqk_d[:])
            # m04[:, 0, :] = diag-block mask (kk<=qq); [:, 1, :] = window
            # edge mask (kk>qq) — applied as one two-region strided op
            m04t = pp.tile([128, 2, 128], bf16, tag="m04")
            nc.sync.dma_start(
                m04t[:], m04_d[:].rearrange("p (a b) -> p a b", a=2)
            )

            qk = [
                pp.tile([128, S], bf16, tag=f"qk{c}", name=f"qk{c}") for c in range(4)
            ]
            CV = NH * 65  # 260: per head 64 v-cols + 1 ones col
            vbig = pp.tile([128, NB, CV], bf16, tag="vbig")
            outH = pp.tile([128, 2, S], bf16, tag="outH")
            wpt = pp.tile([128, 2, D], bf16, tag="wpt")

            wbig = xw_pool.tile([128, KT, 3 * NH * HD], bf16, tag="wbig")
            xbig = xw_pool.tile([128, KT, S], bf16, tag="xbig")
            cos2 = xw_pool.tile([128, S], bf16, tag="cos2")
            sin2 = xw_pool.tile([128, S], bf16, tag="sin2")

            # DMA order = need order: v-cols, x cols 0:1024, qk-cols,
            # rope tables, x cols 1024:2048, c_proj weights. x moves in
            # 1024-col slices (2 KB contiguous lines) for DMA efficiency.
            VC = 2 * NH * HD
            for kt in range(KT):
                nc.sync.dma_start(wbig[:, kt, VC:], wqkv_d[ts(kt, 128), VC:])
            for kt in range(KT):
                nc.sync.dma_start(xbig[:, kt, 0:1024], xT_d[ts(kt, 128), 0:1024])
            for kt in range(KT):
                nc.sync.dma_start(wbig[:, kt, 0:VC], wqkv_d[ts(kt, 128), 0:VC])
            nc.sync.dma_start(cos2[:], cos2_d[:])
            nc.sync.dma_start(sin2[:], sin2_d[:])
            for kt in range(KT):
                nc.sync.dma_start(
                    xbig[:, kt, 1024:2048], xT_d[ts(kt, 128), 1024:2048]
                )
            for k2 in range(2):
                nc.sync.dma_start(wpt[:, k2, :], wp_d[ts(k2, 128), :])

            # PE prewarm: keep the tensor engine busy (and the HAM
            # clock-gate open) while the first input DMAs land.
            for w in range(18):
                psw = psj_pool.tile([128, 512], f32, tag="psj", name="psw")
                nc.tensor.matmul(
                    psw[:], zb[:, 0:128], zb[:, 0:512],
                    start=True, stop=True,
                )

            # ---------------- building blocks ----------------
            eTs = [dict() for _ in range(NH)]  # [h][kj] -> masked exp'd scores

            def proj_v(sb):
                vsb = vbig[:, sb, :].rearrange("p (h c) -> p h c", c=65)
                nc.vector.memset(vsb[:, :, 64:65], 1.0)
                psv = psj_pool.tile([128, 256], f32, tag="psj", name="psv")
                for kt in range(KT):
                    nc.tensor.matmul(
                        psv[:],
                        xbig[:, kt, ts(sb, 128)],
                        wbig[:, kt, ds(2 * NH * HD, NH * HD)],
                        start=(kt == 0),
                        stop=(kt == KT - 1),
                    )
                nc.vector.tensor_copy(
                    vsb[:, :, 0:64],
                    psv[:].rearrange("p (h c) -> p h c", c=64),
                )

            def proj_qk(c, sc):
                psb = psj_pool.tile([128, 512], f32, tag="psj", name="psb")
                for kt in range(KT):
                    nc.tensor.matmul(
                        psb[:],
                        wbig[:, kt, ts(c, 128)],
                        xbig[:, kt, ts(sc, 512)],
                        start=(kt == 0),
                        stop=(kt == KT - 1),
                    )
                nc.scalar.add(
                    qk[c][:, ts(sc, 512)], psb[:], bqk_t[:, c : c + 1]
                )
                # rope: rotate_half via partition-swap SBUF DMAs on the
                # sync/gpsimd queues (sign is folded into the sin table)
                qc = qk[c][:, ts(sc, 512)]
                tmp = tmp_pool.tile([128, 512], bf16, tag="ropetmp", name="tmp")
                dma_engs = [nc.sync, nc.gpsimd, nc.sync, nc.gpsimd]
                for g in range(2):
                    b0 = g * 64
                    dma_engs[2 * g].dma_start(
                        tmp[b0 : b0 + 32, :],
                        qk[c][b0 + 32 : b0 + 64, ts(sc, 512)],
                    )
                    dma_engs[2 * g + 1].dma_start(
                        tmp[b0 + 32 : b0 + 64, :],
                        qk[c][b0 : b0 + 32, ts(sc, 512)],
                    )
                nc.vector.tensor_tensor(
                    tmp[:], tmp[:], sin2[:, ts(sc, 512)], op=mult
                )
                nc.vector.tensor_tensor(qc, qc, cos2[:, ts(sc, 512)], op=mult)
                nc.vector.tensor_add(qc, qc, tmp[:])

            def scores_exp(h, kj):
                # transposed scores sT[k, q] for the full band of kj
                # (5 q-blocks), exp'd on ACT, masked on DVE
                hb = (h % 2) * 64
                qt = qk[h // 2]
                kt_ = qk[2 + h // 2]
                nq = min(WB + 1, NB - kj)
                pss = ps_pool.tile([128, 640], f32, tag="ps", name="pss")
                n1 = min(512, nq * 128)
                n2 = nq * 128 - n1
                lhs_k = kt_[hb : hb + 64, ts(kj, 128)]
                nc.tensor.matmul(
                    pss[:, 0:n1],
                    lhs_k,
                    qt[hb : hb + 64, ds(kj * 128, n1)],
                    start=True,
                    stop=True,
                )
                if n2:
                    nc.tensor.matmul(
                        pss[:, 512 : 512 + n2],
                        lhs_k,
                        qt[hb : hb + 64, ds(kj * 128 + 512, n2)],
                        start=True,
                        stop=True,
                    )
                return pss, nq

            def exp_mask(h, kj, pss, nq):
                eT = e_pool.tile([128, 640], bf16, tag="et", name="eT")
                nc.scalar.activation(
                    eT[:, 0 : nq * 128], pss[:, 0 : nq * 128], Exp, scale=0.125
                )
                # banded mask: diag block keeps kk<=qq, window edge kk>qq.
                # Both 128-col regions are masked in one strided DVE op.
                if nq == WB + 1:
                    e2 = eT[:, 0:640].rearrange("p (x y) -> p x y", y=128)[
                        :, 0:5:4, :
                    ]
                    nc.vector.tensor_tensor(e2, e2, m04t[:], op=mult)
                else:
                    nc.vector.tensor_tensor(
                        eT[:, 0:128], eT[:, 0:128], m04t[:, 0, :], op=mult
                    )
                eTs[h][kj] = eT

            def evac_q(po, h, qtr):
                # normalize by denominators (PSUM row 64) into outH.
                # custom-DVE must not read PSUM: stage denom row via ACT.
                hb = (h % 2) * 64
                rb = rb_pool.tile([64, HS], f32, tag="rb", name="rb")
                nc.scalar.copy(rb[0:1, :], po[64:65, :])
                nc.vector.reciprocal_approx_fast(rb[0:1, :], rb[0:1, :])
                nc.gpsimd.partition_broadcast(rb[:], rb[0:1, :])
                nc.vector.tensor_tensor(
                    outH[hb : hb + 64, h // 2, qtr * HS : (qtr + 1) * HS],
                    po[0:64, :],
                    rb[:],
                    op=mult,
                )

            def attnv(h, qtr):
                # kj-major attn@V: each V block loads once and streams its
                # whole q-span (per-element has_written bits handle the
                # staggered accumulation regions)
                kjlo = max(0, 4 * qtr - WB)
                kjhi = 4 * qtr + QB - 1
                po = pso_pool.tile([65, HS], f32, tag="pso", name="po")
                for kj in range(kjlo, kjhi + 1):
                    qlo = max(4 * qtr, kj)
                    qhi = min(4 * qtr + QB - 1, kj + WB)
                    off = (qlo - kj) * 128
                    n = (qhi - qlo + 1) * 128
                    nc.tensor.matmul(
                        po[:, ds((qlo - 4 * qtr) * 128, n)],
                        vbig[:, kj, h * 65 : h * 65 + 65],
                        eTs[h][kj][:, off : off + n],
                        start=(kj == kjlo),
                        stop=(kj == kjhi),
                    )
                evac_q(po, h, qtr)

            def cproj(sb):
                psp = ps_pool.tile([128, 1024], f32, tag="ps", name="psp")
                for k2 in range(2):
                    for ncol in range(2):
                        nc.tensor.matmul(
                            psp[:, ts(ncol, 512)],
                            outH[:, k2, ts(sb, 128)],
                            wpt[:, k2, ts(ncol, 512)],
                            start=(k2 == 0),
                            stop=(k2 == 1),
                        )
                yt = y_pool.tile([128, D], bf16, tag="yo", name="yt")
                if sb % 2 == 0:
                    nc.scalar.copy(yt[:], psp[:])
                else:
                    nc.vector.tensor_copy(yt[:], psp[:])
                nc.sync.dma_start(out_d[ts(sb, 128), :], yt[:])

            def wave(qtr, chunk=None):
                # quarter qtr's attention, with chunk's projection work
                # woven between its items so the PE stays busy while ACT
                # crunches the exps the attention items depend on.
                vs = list(range(4 * chunk, 4 * chunk + 4)) if chunk is not None else []
                cs = [(c, chunk) for c in (0, 2, 1, 3)] if chunk is not None else []
                for i, kj in enumerate(range(4 * qtr, 4 * qtr + 4)):
                    for pair in range(2):
                        s0 = scores_exp(2 * pair, kj)
                        s1 = scores_exp(2 * pair + 1, kj)
                        exp_mask(2 * pair, kj, *s0)
                        exp_mask(2 * pair + 1, kj, *s1)
                    if i < len(vs):
                        proj_v(vs[i])
                for h in range(NH):
                    attnv(h, qtr)
                    if h < len(cs):
                        proj_qk(*cs[h])
                for sb in range(4 * qtr, 4 * qtr + 4):
                    cproj(sb)

            # ---------------- pipelined schedule ----------------
            for sc in range(2):
                for sb in range(4 * sc, 4 * sc + 4):
                    proj_v(sb)
                for c in (0, 2, 1, 3):
                    proj_qk(c, sc)
            wave(0, 2)
            wave(1, 3)
            wave(2)
            wave(3)

    nc.compile()
    return nc


def _host_inputs(hidden, pos, caw, cab, cpw):
    """Build the 8 per-core input maps."""
    inv = 1.0 / (ROPE_BASE ** (np.arange(0, HD, 2, dtype=np.float32) / HD))
    t = np.arange(S, dtype=np.float32)
    freqs = np.outer(t, inv).astype(np.float32)
    emb = np.concatenate([freqs, freqs], axis=1)  # [S, HD]
    cos = np.cos(emb).astype(np.float32)
    sin = np.sin(emb).astype(np.float32)

    import ml_dtypes

    bf = ml_dtypes.bfloat16
    ii = np.arange(128)
    m0 = (ii[:, None] <= ii[None, :]).astype(bf)
    m4 = (ii[:, None] > ii[None, :]).astype(bf)
    m04 = np.ascontiguousarray(np.concatenate([m0, m4], axis=1))

    xTs, cos2s, sin2s = [], [], []
    for b in range(B):
        xTs.append(np.ascontiguousarray(hidden[b].T).astype(bf))
        cosT = np.ascontiguousarray(cos[pos[b]].T)  # [HD, S]
        sinT = np.ascontiguousarray(sin[pos[b]].T)
        sinS = np.concatenate([-sinT[:32], sinT[32:]], axis=0)
        cos2s.append(np.tile(cosT, (2, 1)).astype(bf))
        sin2s.append(np.tile(sinS, (2, 1)).astype(bf))

    in_maps = []
    for c in range(NCORES):
        b = c // 4
        h0 = NH * (c % 4)
        col = h0 * HD
        w_q = caw[:, col : col + NH * HD]
        w_k = caw[:, D + col : D + col + NH * HD]
        w_v = caw[:, 2 * D + col : 2 * D + col + NH * HD]
        wqkv = np.ascontiguousarray(
            np.concatenate([w_q, w_k, w_v], axis=1)
        ).astype(bf)
        b_q = cab[col : col + NH * HD]
        b_k = cab[D + col : D + col + NH * HD]
        bqk = np.ascontiguousarray(
            np.concatenate([b_q, b_k]).reshape(4, 128).T
        )  # [128, 4]: partition = col within tile
        wp = np.ascontiguousarray(cpw[col : col + NH * HD, :]).astype(bf)
        in_maps.append(
            {
                "xT": xTs[b],
                "wqkv": wqkv,
                "bqk": bqk,
                "wp": wp,
                "cos2": cos2s[b],
                "sin2": sin2s[b],
                "m04": m04,
            }
        )
    return in_maps


def _assemble(results, cab, cpw, cpb):
    """Host all-reduce of the 4 per-batch partials + biases.

    The V-bias contribution is position-independent after softmax
    (attn rows sum to 1), so it folds into a constant row vector:
    bias_v @ c_proj_w.
    """
    vrow = cab[2 * D :].astype(np.float32) @ cpw.astype(np.float32)
    bias = cpb.astype(np.float32) + vrow
    y = np.empty((B, S, D), dtype=np.float32)
    for b in range(B):
        acc = results[4 * b]["out"].astype(np.float32)
        for c in range(4 * b + 1, 4 * b + 4):
            acc = acc + results[c]["out"].astype(np.float32)
        y[b] = acc + bias[None, :]
    return y


def kernel(**inputs):
    from concourse import bass_utils

    hidden = np.asarray(inputs["hidden_states"], dtype=np.float32)
    pos = np.asarray(inputs["position_ids"]).astype(np.int64)
    caw = np.asarray(inputs["c_attn_w"], dtype=np.float32)
    cab = np.asarray(inputs["c_attn_b"], dtype=np.float32)
    cpw = np.asarray(inputs["c_proj_w"], dtype=np.float32)
    cpb = np.asarray(inputs["c_proj_b"], dtype=np.float32)

    in_maps = _host_inputs(hidden, pos, caw, cab, cpw)
    nc = _build_nc()
    res = bass_utils.run_bass_kernel_spmd(nc, in_maps, list(range(NCORES)))
    return _assemble(res.results, cab, cpw, cpb)


# revision 19
# speedup vs baseline: 1.7928x; 1.1473x over previous
"""Trainium2 Bass kernel: GPT2-style windowed attention (DecisionTransformer).

Full-input contract: kernel(**inputs) -> [B, S, D] float32.

Sharding: batch*heads across 8 cores (core c -> batch c//4, heads 4*(c%4)..+4).
Each core: column-sliced c_attn, full windowed attention for its 4 heads,
row-sliced c_proj producing a partial [S, D] output; host sums partials
(the "all-reduce") and adds c_proj bias + V-bias contribution once.

Layout / schedule choices:
  - all matmul operands are bf16 (tolerance is 2e-2; bf16 keeps PE at
    1 cyc/row and halves HBM traffic). PSUM accumulation stays fp32.
  - hidden is sent pre-transposed (xT [D, S]) so QK projections emit
    qT/kT directly in [head*dim, seq] layout; V is projected in
    [seq, head*dim] layout with a ones-column per head so attn@V
    accumulates softmax denominators in PSUM row 64 for free. V-bias
    folds into the host-side reduce (softmax rows sum to 1).
  - the whole kernel is software-pipelined: projections advance per
    512-col seq chunk, and attention q-quarters (scores+exp for all 4
    heads, kj-major attn@V, normalize, c_proj slice, output DMA) are
    issued as soon as their chunk dependencies are met. ACT does only
    exp during attention (the gating engine); evacuations go to DVE.
  - rope: rotate_half via 4 cross-quadrant 32-partition copies (2 DVE,
    2 GPSIMD) -- no DMA, no partition-swap latency chain.
  - scores for a head pair (partitions 0-63 / 64-127, K=64 each) are
    issued back-to-back so the PE runs them as concurrent row-tiles.
  - a dozen dummy matmuls at t=0 keep the PE busy (and the HAM
    clock-gate open) while the first input DMAs land.
"""

import sys

import numpy as np

sys.path.insert(0, "/opt/trn_rl_repo")

B, S, D = 2, 2048, 1024
H, HD = 16, 64
WINDOW = 512
ROPE_BASE = 4000.0
NCORES = 8
NH = 4          # heads per core
KT = D // 128   # 8 contraction tiles for c_attn
NB = S // 128   # 16 seq blocks
WB = WINDOW // 128  # 4 -> band spans up to 5 query blocks per key block


def _build_nc(debug_taps=False):
    import concourse.bass as bass
    from concourse import bacc, library_config, mybir
    import concourse.tile as tile

    f32 = mybir.dt.float32
    bf16 = mybir.dt.bfloat16
    Exp = mybir.ActivationFunctionType.Exp
    mult = mybir.AluOpType.mult
    ts = bass.ts
    ds = bass.ds

    nc = bacc.Bacc("TRN2")

    xT_d = nc.dram_tensor("xT", [D, S], bf16, kind="ExternalInput")
    wqkv_d = nc.dram_tensor("wqkv", [D, 3 * NH * HD], bf16, kind="ExternalInput")
    bqk_d = nc.dram_tensor("bqk", [128, 4], f32, kind="ExternalInput")
    wp_d = nc.dram_tensor("wp", [NH * HD, D], bf16, kind="ExternalInput")
    cos2_d = nc.dram_tensor("cos2", [128, S], bf16, kind="ExternalInput")
    sin2_d = nc.dram_tensor("sin2", [128, S], bf16, kind="ExternalInput")
    m04_d = nc.dram_tensor("m04", [128, 256], bf16, kind="ExternalInput")
    out_d = nc.dram_tensor("out", [S, D], bf16, kind="ExternalOutput")

    HS = S // 4  # q-quarter span: po is 1 PSUM bank
    QB = NB // 4  # 4 q-blocks per quarter

    with tile.TileContext(nc) as tc:
        nc.gpsimd.load_library(library_config.attn)

        with (
            tc.tile_pool(name="persist", bufs=1) as pp,
            tc.tile_pool(name="psj", bufs=2, space="PSUM") as psj_pool,
            tc.tile_pool(name="ps", bufs=2, space="PSUM") as ps_pool,
            tc.tile_pool(name="pso", bufs=2, space="PSUM") as pso_pool,
            tc.tile_pool(name="xw", bufs=1) as xw_pool,
            tc.tile_pool(name="ropetmp", bufs=4) as tmp_pool,
            tc.tile_pool(name="et", bufs=36) as e_pool,
            tc.tile_pool(name="rb", bufs=3) as rb_pool,
            tc.tile_pool(name="yo", bufs=3) as y_pool,
        ):
            # prewarm tile: zeros, matmul'd while input DMAs land
            zb = pp.tile([128, 512], bf16, tag="zb")
            nc.vector.memset(zb[:], 0.0)

            bqk_t = pp.tile([128, 4], f32, tag="bqk")
            nc.sync.dma_start(bqk_t[:], bqk_d[:])
            # m04[:, 0, :] = diag-block mask (kk<=qq); [:, 1, :] = window
            # edge mask (kk>qq) — applied as one two-region strided op
            m04t = pp.tile([128, 2, 128], bf16, tag="m04")
            nc.sync.dma_start(
                m04t[:], m04_d[:].rearrange("p (a b) -> p a b", a=2)
            )

            qk = [
                pp.tile([128, S], bf16, tag=f"qk{c}", name=f"qk{c}") for c in range(4)
            ]
            CV = NH * 65  # 260: per head 64 v-cols + 1 ones col
            vbig = pp.tile([128, NB, CV], bf16, tag="vbig")
            outH = pp.tile([128, 2, S], bf16, tag="outH")
            wpt = pp.tile([128, 2, D], bf16, tag="wpt")

            wbig = xw_pool.tile([128, KT, 3 * NH * HD], bf16, tag="wbig")
            xbig = xw_pool.tile([128, KT, S], bf16, tag="xbig")
            cos2 = xw_pool.tile([128, S], bf16, tag="cos2")
            sin2 = xw_pool.tile([128, S], bf16, tag="sin2")

            # DMA order = need order: v-cols, x first chunk, qk-cols, rope
            # tables, rest of x, c_proj weights.
            VC = 2 * NH * HD
            for kt in range(KT):
                nc.sync.dma_start(wbig[:, kt, VC:], wqkv_d[ts(kt, 128), VC:])
            for kt in range(KT):
                nc.sync.dma_start(xbig[:, kt, 0:512], xT_d[ts(kt, 128), 0:512])
            for kt in range(KT):
                nc.sync.dma_start(wbig[:, kt, 0:VC], wqkv_d[ts(kt, 128), 0:VC])
            nc.sync.dma_start(cos2[:], cos2_d[:])
            nc.sync.dma_start(sin2[:], sin2_d[:])
            for kt in range(KT):
                nc.sync.dma_start(xbig[:, kt, 512:1024], xT_d[ts(kt, 128), 512:1024])
            for kt in range(KT):
                nc.sync.dma_start(
                    xbig[:, kt, 1024:2048], xT_d[ts(kt, 128), 1024:2048]
                )
            for k2 in range(2):
                nc.sync.dma_start(wpt[:, k2, :], wp_d[ts(k2, 128), :])

            # PE prewarm: keep the tensor engine busy (and the HAM
            # clock-gate open) while the first input DMAs land.
            for w in range(14):
                psw = psj_pool.tile([128, 512], f32, tag="psj", name="psw")
                nc.tensor.matmul(
                    psw[:], zb[:, 0:128], zb[:, 0:512],
                    start=True, stop=True,
                )

            # ---------------- building blocks ----------------
            eTs = [dict() for _ in range(NH)]  # [h][kj] -> masked exp'd scores

            def proj_v(sb):
                vsb = vbig[:, sb, :].rearrange("p (h c) -> p h c", c=65)
                nc.vector.memset(vsb[:, :, 64:65], 1.0)
                psv = psj_pool.tile([128, 256], f32, tag="psj", name="psv")
                for kt in range(KT):
                    nc.tensor.matmul(
                        psv[:],
                        xbig[:, kt, ts(sb, 128)],
                        wbig[:, kt, ds(2 * NH * HD, NH * HD)],
                        start=(kt == 0),
                        stop=(kt == KT - 1),
                    )
                nc.vector.tensor_copy(
                    vsb[:, :, 0:64],
                    psv[:].rearrange("p (h c) -> p h c", c=64),
                )

            def proj_qk(c, sc):
                psb = psj_pool.tile([128, 512], f32, tag="psj", name="psb")
                for kt in range(KT):
                    nc.tensor.matmul(
                        psb[:],
                        wbig[:, kt, ts(c, 128)],
                        xbig[:, kt, ts(sc, 512)],
                        start=(kt == 0),
                        stop=(kt == KT - 1),
                    )
                nc.scalar.add(
                    qk[c][:, ts(sc, 512)], psb[:], bqk_t[:, c : c + 1]
                )
                # rope: rotate_half via partition-swap SBUF DMAs on the
                # sync/gpsimd queues (sign is folded into the sin table)
                qc = qk[c][:, ts(sc, 512)]
                tmp = tmp_pool.tile([128, 512], bf16, tag="ropetmp", name="tmp")
                dma_engs = [nc.sync, nc.gpsimd, nc.sync, nc.gpsimd]
                for g in range(2):
                    b0 = g * 64
                    dma_engs[2 * g].dma_start(
                        tmp[b0 : b0 + 32, :],
                        qk[c][b0 + 32 : b0 + 64, ts(sc, 512)],
                    )
                    dma_engs[2 * g + 1].dma_start(
                        tmp[b0 + 32 : b0 + 64, :],
                        qk[c][b0 : b0 + 32, ts(sc, 512)],
                    )
                nc.vector.tensor_tensor(
                    tmp[:], tmp[:], sin2[:, ts(sc, 512)], op=mult
                )
                nc.vector.tensor_tensor(qc, qc, cos2[:, ts(sc, 512)], op=mult)
                nc.vector.tensor_add(qc, qc, tmp[:])

            def scores_exp(h, kj):
                # transposed scores sT[k, q] for the full band of kj
                # (5 q-blocks), exp'd on ACT, masked on DVE
                hb = (h % 2) * 64
                qt = qk[h // 2]
                kt_ = qk[2 + h // 2]
                nq = min(WB + 1, NB - kj)
                pss = ps_pool.tile([128, 640], f32, tag="ps", name="pss")
                n1 = min(512, nq * 128)
                n2 = nq * 128 - n1
                lhs_k = kt_[hb : hb + 64, ts(kj, 128)]
                nc.tensor.matmul(
                    pss[:, 0:n1],
                    lhs_k,
                    qt[hb : hb + 64, ds(kj * 128, n1)],
                    start=True,
                    stop=True,
                )
                if n2:
                    nc.tensor.matmul(
                        pss[:, 512 : 512 + n2],
                        lhs_k,
                        qt[hb : hb + 64, ds(kj * 128 + 512, n2)],
                        start=True,
                        stop=True,
                    )
                return pss, nq

            def exp_mask(h, kj, pss, nq):
                eT = e_pool.tile([128, 640], bf16, tag="et", name="eT")
                nc.scalar.activation(
                    eT[:, 0 : nq * 128], pss[:, 0 : nq * 128], Exp, scale=0.125
                )
                # banded mask: diag block keeps kk<=qq, window edge kk>qq.
                # Both 128-col regions are masked in one strided DVE op.
                if nq == WB + 1:
                    e2 = eT[:, 0:640].rearrange("p (x y) -> p x y", y=128)[
                        :, 0:5:4, :
                    ]
                    nc.vector.tensor_tensor(e2, e2, m04t[:], op=mult)
                else:
                    nc.vector.tensor_tensor(
                        eT[:, 0:128], eT[:, 0:128], m04t[:, 0, :], op=mult
                    )
                eTs[h][kj] = eT

            def evac_q(po, h, qtr):
                # normalize by denominators (PSUM row 64) into outH.
                # custom-DVE must not read PSUM: stage denom row via ACT.
                hb = (h % 2) * 64
                rb = rb_pool.tile([64, HS], f32, tag="rb", name="rb")
                nc.scalar.copy(rb[0:1, :], po[64:65, :])
                nc.vector.reciprocal_approx_fast(rb[0:1, :], rb[0:1, :])
                nc.gpsimd.partition_broadcast(rb[:], rb[0:1, :])
                nc.vector.tensor_tensor(
                    outH[hb : hb + 64, h // 2, qtr * HS : (qtr + 1) * HS],
                    po[0:64, :],
                    rb[:],
                    op=mult,
                )

            def attnv(h, qtr):
                # kj-major attn@V: each V block loads once and streams its
                # whole q-span (per-element has_written bits handle the
                # staggered accumulation regions)
                kjlo = max(0, 4 * qtr - WB)
                kjhi = 4 * qtr + QB - 1
                po = pso_pool.tile([65, HS], f32, tag="pso", name="po")
                for kj in range(kjlo, kjhi + 1):
                    qlo = max(4 * qtr, kj)
                    qhi = min(4 * qtr + QB - 1, kj + WB)
                    off = (qlo - kj) * 128
                    n = (qhi - qlo + 1) * 128
                    nc.tensor.matmul(
                        po[:, ds((qlo - 4 * qtr) * 128, n)],
                        vbig[:, kj, h * 65 : h * 65 + 65],
                        eTs[h][kj][:, off : off + n],
                        start=(kj == kjlo),
                        stop=(kj == kjhi),
                    )
                evac_q(po, h, qtr)

            def cproj(sb):
                psp = ps_pool.tile([128, 1024], f32, tag="ps", name="psp")
                for k2 in range(2):
                    for ncol in range(2):
                        nc.tensor.matmul(
                            psp[:, ts(ncol, 512)],
                            outH[:, k2, ts(sb, 128)],
                            wpt[:, k2, ts(ncol, 512)],
                            start=(k2 == 0),
                            stop=(k2 == 1),
                        )
                yt = y_pool.tile([128, D], bf16, tag="yo", name="yt")
                if sb % 2 == 0:
                    nc.scalar.copy(yt[:], psp[:])
                else:
                    nc.vector.tensor_copy(yt[:], psp[:])
                nc.sync.dma_start(out_d[ts(sb, 128), :], yt[:])

            def wave(qtr, chunk=None, cpq=None):
                # quarter qtr's attention, with filler work (chunk's
                # projections, previous quarter's c_proj) woven between its
                # items so the PE stays busy (and the clock-gate warm)
                # while ACT crunches the exps the attention items depend
                # on, and so ACT evacuations interleave with exps instead
                # of queueing behind a full wave of them.
                fill = []
                if chunk is not None:
                    for j in range(4):
                        fill.append(("v", 4 * chunk + j))
                        fill.append(("qk", (0, 2, 1, 3)[j], chunk))
                if cpq is not None:
                    fill.extend(("cp", 4 * cpq + j) for j in range(4))

                def filler(i):
                    if i < len(fill):
                        it = fill[i]
                        if it[0] == "v":
                            proj_v(it[1])
                        elif it[0] == "qk":
                            proj_qk(it[1], it[2])
                        else:
                            cproj(it[1])

                for i, kj in enumerate(range(4 * qtr, 4 * qtr + 4)):
                    for pair in range(2):
                        s0 = scores_exp(2 * pair, kj)
                        s1 = scores_exp(2 * pair + 1, kj)
                        exp_mask(2 * pair, kj, *s0)
                        exp_mask(2 * pair + 1, kj, *s1)
                        filler(2 * i + pair)
                for h in range(NH):
                    attnv(h, qtr)
                    filler(8 + h)

            # ---------------- pipelined schedule ----------------
            for sc in range(2):
                for sb in range(4 * sc, 4 * sc + 4):
                    proj_v(sb)
                for c in (0, 2, 1, 3):
                    proj_qk(c, sc)
            wave(0, chunk=2)
            wave(1, chunk=3, cpq=0)
            wave(2, cpq=1)
            wave(3, cpq=2)
            for sb in range(12, 16):
                cproj(sb)

    nc.compile()
    return nc


def _host_inputs(hidden, pos, caw, cab, cpw):
    """Build the 8 per-core input maps."""
    inv = 1.0 / (ROPE_BASE ** (np.arange(0, HD, 2, dtype=np.float32) / HD))
    t = np.arange(S, dtype=np.float32)
    freqs = np.outer(t, inv).astype(np.float32)
    emb = np.concatenate([freqs, freqs], axis=1)  # [S, HD]
    cos = np.cos(emb).astype(np.float32)
    sin = np.sin(emb).astype(np.float32)

    import ml_dtypes

    bf = ml_dtypes.bfloat16
    ii = np.arange(128)
    m0 = (ii[:, None] <= ii[None, :]).astype(bf)
    m4 = (ii[:, None] > ii[None, :]).astype(bf)
    m04 = np.ascontiguousarray(np.concatenate([m0, m4], axis=1))

    xTs, cos2s, sin2s = [], [], []
    for b in range(B):
        xTs.append(np.ascontiguousarray(hidden[b].T).astype(bf))
        cosT = np.ascontiguousarray(cos[pos[b]].T)  # [HD, S]
        sinT = np.ascontiguousarray(sin[pos[b]].T)
        sinS = np.concatenate([-sinT[:32], sinT[32:]], axis=0)
        cos2s.append(np.tile(cosT, (2, 1)).astype(bf))
        sin2s.append(np.tile(sinS, (2, 1)).astype(bf))

    in_maps = []
    for c in range(NCORES):
        b = c // 4
        h0 = NH * (c % 4)
        col = h0 * HD
        w_q = caw[:, col : col + NH * HD]
        w_k = caw[:, D + col : D + col + NH * HD]
        w_v = caw[:, 2 * D + col : 2 * D + col + NH * HD]
        wqkv = np.ascontiguousarray(
            np.concatenate([w_q, w_k, w_v], axis=1)
        ).astype(bf)
        b_q = cab[col : col + NH * HD]
        b_k = cab[D + col : D + col + NH * HD]
        bqk = np.ascontiguousarray(
            np.concatenate([b_q, b_k]).reshape(4, 128).T
        )  # [128, 4]: partition = col within tile
        wp = np.ascontiguousarray(cpw[col : col + NH * HD, :]).astype(bf)
        in_maps.append(
            {
                "xT": xTs[b],
                "wqkv": wqkv,
                "bqk": bqk,
                "wp": wp,
                "cos2": cos2s[b],
                "sin2": sin2s[b],
                "m04": m04,
            }
        )
    return in_maps


def _assemble(results, cab, cpw, cpb):
    """Host all-reduce of the 4 per-batch partials + biases.

    The V-bias contribution is position-independent after softmax
    (attn rows sum to 1), so it folds into a constant row vector:
    bias_v @ c_proj_w.
    """
    vrow = cab[2 * D :].astype(np.float32) @ cpw.astype(np.float32)
    bias = cpb.astype(np.float32) + vrow
    y = np.empty((B, S, D), dtype=np.float32)
    for b in range(B):
        acc = results[4 * b]["out"].astype(np.float32)
        for c in range(4 * b + 1, 4 * b + 4):
            acc = acc + results[c]["out"].astype(np.float32)
        y[b] = acc + bias[None, :]
    return y


def kernel(**inputs):
    from concourse import bass_utils

    hidden = np.asarray(inputs["hidden_states"], dtype=np.float32)
    pos = np.asarray(inputs["position_ids"]).astype(np.int64)
    caw = np.asarray(inputs["c_attn_w"], dtype=np.float32)
    cab = np.asarray(inputs["c_attn_b"], dtype=np.float32)
    cpw = np.asarray(inputs["c_proj_w"], dtype=np.float32)
    cpb = np.asarray(inputs["c_proj_b"], dtype=np.float32)

    in_maps = _host_inputs(hidden, pos, caw, cab, cpw)
    nc = _build_nc()
    res = bass_utils.run_bass_kernel_spmd(nc, in_maps, list(range(NCORES)))
    return _assemble(res.results, cab, cpw, cpb)


# revision 21
# speedup vs baseline: 1.8647x; 1.0401x over previous
"""Trainium2 Bass kernel: GPT2-style windowed attention (DecisionTransformer).

Full-input contract: kernel(**inputs) -> [B, S, D] float32.

Sharding: batch*heads across 8 cores (core c -> batch c//4, heads 4*(c%4)..+4).
Each core: column-sliced c_attn, full windowed attention for its 4 heads,
row-sliced c_proj producing a partial [S, D] output; host sums partials
(the "all-reduce") and adds c_proj bias + V-bias contribution once.

Layout / schedule choices:
  - all matmul operands are bf16 (tolerance is 2e-2; bf16 keeps PE at
    1 cyc/row and halves HBM traffic). PSUM accumulation stays fp32.
  - hidden is sent pre-transposed (xT [D, S]) so QK projections emit
    qT/kT directly in [head*dim, seq] layout; V is projected in
    [seq, head*dim] layout with a ones-column per head so attn@V
    accumulates softmax denominators in PSUM row 64 for free. V-bias
    folds into the host-side reduce (softmax rows sum to 1).
  - the whole kernel is software-pipelined: projections advance per
    512-col seq chunk, and attention q-quarters (scores+exp for all 4
    heads, kj-major attn@V, normalize, c_proj slice, output DMA) are
    issued as soon as their chunk dependencies are met. ACT does only
    exp during attention (the gating engine); evacuations go to DVE.
  - rope: rotate_half via 4 cross-quadrant 32-partition copies (2 DVE,
    2 GPSIMD) -- no DMA, no partition-swap latency chain.
  - scores for a head pair (partitions 0-63 / 64-127, K=64 each) are
    issued back-to-back so the PE runs them as concurrent row-tiles.
  - a dozen dummy matmuls at t=0 keep the PE busy (and the HAM
    clock-gate open) while the first input DMAs land.
"""

import sys

import numpy as np

sys.path.insert(0, "/opt/trn_rl_repo")

B, S, D = 2, 2048, 1024
H, HD = 16, 64
WINDOW = 512
ROPE_BASE = 4000.0
NCORES = 8
NH = 4          # heads per core
KT = D // 128   # 8 contraction tiles for c_attn
NB = S // 128   # 16 seq blocks
WB = WINDOW // 128  # 4 -> band spans up to 5 query blocks per key block


def _build_nc(debug_taps=False):
    import concourse.bass as bass
    from concourse import bacc, library_config, mybir
    import concourse.tile as tile

    f32 = mybir.dt.float32
    bf16 = mybir.dt.bfloat16
    Exp = mybir.ActivationFunctionType.Exp
    mult = mybir.AluOpType.mult
    ts = bass.ts
    ds = bass.ds

    nc = bacc.Bacc("TRN2")

    xT_d = nc.dram_tensor("xT", [D, S], bf16, kind="ExternalInput")
    wqkv_d = nc.dram_tensor("wqkv", [D, 3 * NH * HD], bf16, kind="ExternalInput")
    bqk_d = nc.dram_tensor("bqk", [128, 4], f32, kind="ExternalInput")
    wp_d = nc.dram_tensor("wp", [NH * HD, D], bf16, kind="ExternalInput")
    cos2_d = nc.dram_tensor("cos2", [128, S], bf16, kind="ExternalInput")
    sin2_d = nc.dram_tensor("sin2", [128, S], bf16, kind="ExternalInput")
    m04_d = nc.dram_tensor("m04", [128, 256], bf16, kind="ExternalInput")
    out_d = nc.dram_tensor("out", [S, D], bf16, kind="ExternalOutput")

    HS = S // 4  # q-quarter span: po is 1 PSUM bank
    QB = NB // 4  # 4 q-blocks per quarter

    with tile.TileContext(nc) as tc:
        nc.gpsimd.load_library(library_config.attn)

        with (
            tc.tile_pool(name="persist", bufs=1) as pp,
            tc.tile_pool(name="psj", bufs=2, space="PSUM") as psj_pool,
            tc.tile_pool(name="ps", bufs=2, space="PSUM") as ps_pool,
            tc.tile_pool(name="pso", bufs=2, space="PSUM") as pso_pool,
            tc.tile_pool(name="xw", bufs=1) as xw_pool,
            tc.tile_pool(name="ropetmp", bufs=4) as tmp_pool,
            tc.tile_pool(name="et", bufs=36) as e_pool,
            tc.tile_pool(name="rb", bufs=3) as rb_pool,
            tc.tile_pool(name="yo", bufs=3) as y_pool,
        ):
            # prewarm tile: zeros, matmul'd while input DMAs land
            zb = pp.tile([128, 512], bf16, tag="zb")
            nc.vector.memset(zb[:], 0.0)

            bqk_t = pp.tile([128, 4], f32, tag="bqk")
            nc.sync.dma_start(bqk_t[:], bqk_d[:])
            # m04[:, 0, :] = diag-block mask (kk<=qq); [:, 1, :] = window
            # edge mask (kk>qq) — applied as one two-region strided op
            m04t = pp.tile([128, 2, 128], bf16, tag="m04")
            nc.sync.dma_start(
                m04t[:], m04_d[:].rearrange("p (a b) -> p a b", a=2)
            )

            qk = [
                pp.tile([128, S], bf16, tag=f"qk{c}", name=f"qk{c}") for c in range(4)
            ]
            CV = NH * 65  # 260: per head 64 v-cols + 1 ones col
            vbig = pp.tile([128, NB, CV], bf16, tag="vbig")
            outH = pp.tile([128, 2, S], bf16, tag="outH")
            wpt = pp.tile([128, 2, D], bf16, tag="wpt")

            wbig = xw_pool.tile([128, KT, 3 * NH * HD], bf16, tag="wbig")
            xbig = xw_pool.tile([128, KT, S], bf16, tag="xbig")
            cos2 = xw_pool.tile([128, S], bf16, tag="cos2")
            sin2 = xw_pool.tile([128, S], bf16, tag="sin2")

            # DMA order = need order: v-cols, x first chunk, qk-cols, rope
            # tables, rest of x, c_proj weights.
            VC = 2 * NH * HD
            for kt in range(KT):
                nc.sync.dma_start(wbig[:, kt, VC:], wqkv_d[ts(kt, 128), VC:])
            for kt in range(KT):
                nc.sync.dma_start(xbig[:, kt, 0:512], xT_d[ts(kt, 128), 0:512])
            for kt in range(KT):
                nc.sync.dma_start(wbig[:, kt, 0:VC], wqkv_d[ts(kt, 128), 0:VC])
            nc.sync.dma_start(cos2[:], cos2_d[:])
            nc.sync.dma_start(sin2[:], sin2_d[:])
            for kt in range(KT):
                nc.sync.dma_start(xbig[:, kt, 512:1024], xT_d[ts(kt, 128), 512:1024])
            for kt in range(KT):
                nc.sync.dma_start(
                    xbig[:, kt, 1024:2048], xT_d[ts(kt, 128), 1024:2048]
                )
            for k2 in range(2):
                nc.sync.dma_start(wpt[:, k2, :], wp_d[ts(k2, 128), :])

            # PE prewarm: keep the tensor engine busy (and the HAM
            # clock-gate open) while the first input DMAs land.
            for w in range(18):
                psw = psj_pool.tile([128, 512], f32, tag="psj", name="psw")
                nc.tensor.matmul(
                    psw[:], zb[:, 0:128], zb[:, 0:512],
                    start=True, stop=True,
                )

            # ---------------- building blocks ----------------
            eTs = [dict() for _ in range(NH)]  # [h][kj] -> masked exp'd scores

            def proj_v(sb):
                vsb = vbig[:, sb, :].rearrange("p (h c) -> p h c", c=65)
                nc.vector.memset(vsb[:, :, 64:65], 1.0)
                psv = psj_pool.tile([128, 256], f32, tag="psj", name="psv")
                for kt in range(KT):
                    nc.tensor.matmul(
                        psv[:],
                        xbig[:, kt, ts(sb, 128)],
                        wbig[:, kt, ds(2 * NH * HD, NH * HD)],
                        start=(kt == 0),
                        stop=(kt == KT - 1),
                    )
                nc.vector.tensor_copy(
                    vsb[:, :, 0:64],
                    psv[:].rearrange("p (h c) -> p h c", c=64),
                )

            def proj_qk(c, sc):
                psb = psj_pool.tile([128, 512], f32, tag="psj", name="psb")
                for kt in range(KT):
                    nc.tensor.matmul(
                        psb[:],
                        wbig[:, kt, ts(c, 128)],
                        xbig[:, kt, ts(sc, 512)],
                        start=(kt == 0),
                        stop=(kt == KT - 1),
                    )
                nc.scalar.add(
                    qk[c][:, ts(sc, 512)], psb[:], bqk_t[:, c : c + 1]
                )
                # rope: rotate_half via partition-swap SBUF DMAs on the
                # sync/gpsimd queues (sign is folded into the sin table)
                qc = qk[c][:, ts(sc, 512)]
                tmp = tmp_pool.tile([128, 512], bf16, tag="ropetmp", name="tmp")
                dma_engs = [nc.sync, nc.gpsimd, nc.sync, nc.gpsimd]
                for g in range(2):
                    b0 = g * 64
                    dma_engs[2 * g].dma_start(
                        tmp[b0 : b0 + 32, :],
                        qk[c][b0 + 32 : b0 + 64, ts(sc, 512)],
                    )
                    dma_engs[2 * g + 1].dma_start(
                        tmp[b0 + 32 : b0 + 64, :],
                        qk[c][b0 : b0 + 32, ts(sc, 512)],
                    )
                nc.vector.tensor_tensor(
                    tmp[:], tmp[:], sin2[:, ts(sc, 512)], op=mult
                )
                nc.vector.tensor_tensor(qc, qc, cos2[:, ts(sc, 512)], op=mult)
                nc.vector.tensor_add(qc, qc, tmp[:])

            def scores_exp(h, kj):
                # transposed scores sT[k, q] for the full band of kj
                # (5 q-blocks), exp'd on ACT, masked on DVE
                hb = (h % 2) * 64
                qt = qk[h // 2]
                kt_ = qk[2 + h // 2]
                nq = min(WB + 1, NB - kj)
                pss = ps_pool.tile([128, 640], f32, tag="ps", name="pss")
                n1 = min(512, nq * 128)
                n2 = nq * 128 - n1
                lhs_k = kt_[hb : hb + 64, ts(kj, 128)]
                nc.tensor.matmul(
                    pss[:, 0:n1],
                    lhs_k,
                    qt[hb : hb + 64, ds(kj * 128, n1)],
                    start=True,
                    stop=True,
                )
                if n2:
                    nc.tensor.matmul(
                        pss[:, 512 : 512 + n2],
                        lhs_k,
                        qt[hb : hb + 64, ds(kj * 128 + 512, n2)],
                        start=True,
                        stop=True,
                    )
                return pss, nq

            def exp_mask(h, kj, pss, nq):
                eT = e_pool.tile([128, 640], bf16, tag="et", name="eT")
                nc.scalar.activation(
                    eT[:, 0 : nq * 128], pss[:, 0 : nq * 128], Exp, scale=0.125
                )
                # banded mask: diag block keeps kk<=qq, window edge kk>qq.
                # Both 128-col regions are masked in one strided DVE op.
                if nq == WB + 1:
                    e2 = eT[:, 0:640].rearrange("p (x y) -> p x y", y=128)[
                        :, 0:5:4, :
                    ]
                    nc.vector.tensor_tensor(e2, e2, m04t[:], op=mult)
                else:
                    nc.vector.tensor_tensor(
                        eT[:, 0:128], eT[:, 0:128], m04t[:, 0, :], op=mult
                    )
                eTs[h][kj] = eT

            def evac_q(po, h, qtr):
                # normalize by denominators (PSUM row 64) into outH.
                # custom-DVE must not read PSUM: stage denom row via ACT.
                hb = (h % 2) * 64
                rb = rb_pool.tile([64, HS], f32, tag="rb", name="rb")
                nc.scalar.copy(rb[0:1, :], po[64:65, :])
                nc.vector.reciprocal_approx_fast(rb[0:1, :], rb[0:1, :])
                nc.gpsimd.partition_broadcast(rb[:], rb[0:1, :])
                nc.vector.tensor_tensor(
                    outH[hb : hb + 64, h // 2, qtr * HS : (qtr + 1) * HS],
                    po[0:64, :],
                    rb[:],
                    op=mult,
                )

            def attnv(h, qtr):
                # kj-major attn@V: each V block loads once and streams its
                # whole q-span (per-element has_written bits handle the
                # staggered accumulation regions)
                kjlo = max(0, 4 * qtr - WB)
                kjhi = 4 * qtr + QB - 1
                po = pso_pool.tile([65, HS], f32, tag="pso", name="po")
                for kj in range(kjlo, kjhi + 1):
                    qlo = max(4 * qtr, kj)
                    qhi = min(4 * qtr + QB - 1, kj + WB)
                    off = (qlo - kj) * 128
                    n = (qhi - qlo + 1) * 128
                    nc.tensor.matmul(
                        po[:, ds((qlo - 4 * qtr) * 128, n)],
                        vbig[:, kj, h * 65 : h * 65 + 65],
                        eTs[h][kj][:, off : off + n],
                        start=(kj == kjlo),
                        stop=(kj == kjhi),
                    )
                evac_q(po, h, qtr)

            def cproj(sb):
                # two 1-bank psum groups so c_proj never contends with the
                # scores pool
                yt = y_pool.tile([128, D], bf16, tag="yo", name="yt")
                for ncol in range(2):
                    psp = psj_pool.tile([128, 512], f32, tag="psj", name="psp")
                    for k2 in range(2):
                        nc.tensor.matmul(
                            psp[:],
                            outH[:, k2, ts(sb, 128)],
                            wpt[:, k2, ts(ncol, 512)],
                            start=(k2 == 0),
                            stop=(k2 == 1),
                        )
                    if (sb + ncol) % 2 == 0:
                        nc.scalar.copy(yt[:, ts(ncol, 512)], psp[:])
                    else:
                        nc.vector.tensor_copy(yt[:, ts(ncol, 512)], psp[:])
                nc.sync.dma_start(out_d[ts(sb, 128), :], yt[:])

            def scores_q(qtr):
                # scores+exp for a whole quarter, pair-interleaved so the
                # two heads' K=64 matmuls run as concurrent PE row-tiles
                for kj in range(4 * qtr, 4 * qtr + 4):
                    for pair in range(2):
                        s0 = scores_exp(2 * pair, kj)
                        s1 = scores_exp(2 * pair + 1, kj)
                        exp_mask(2 * pair, kj, *s0)
                        exp_mask(2 * pair + 1, kj, *s1)

            def wave(qtr):
                # Scores run a full wave ahead: this wave consumes quarter
                # qtr's eT tiles (exp'd at the end of wave qtr-1) and
                # produces quarter qtr+1's. Projection chunks / previous
                # quarter's c_proj lead the wave so the PE stays dense
                # (and the clock-gate warm) while ACT drains the exp
                # backlog; two attn@V heads run before the new scores and
                # two after to bracket the exp burst with PE work.
                ch = qtr + 2
                if ch < 4:
                    for j in range(4):
                        proj_v(4 * ch + j)
                        proj_qk((0, 2, 1, 3)[j], ch)
                if qtr >= 1:
                    for j in range(4):
                        cproj(4 * (qtr - 1) + j)
                attnv(0, qtr)
                attnv(1, qtr)
                if qtr < 3:
                    scores_q(qtr + 1)
                attnv(2, qtr)
                attnv(3, qtr)

            # ---------------- pipelined schedule ----------------
            for sc in range(2):
                for sb in range(4 * sc, 4 * sc + 4):
                    proj_v(sb)
                for c in (0, 2, 1, 3):
                    proj_qk(c, sc)
            scores_q(0)
            for qtr in range(4):
                wave(qtr)
            for sb in range(12, 16):
                cproj(sb)

    nc.compile()
    return nc


def _host_inputs(hidden, pos, caw, cab, cpw):
    """Build the 8 per-core input maps."""
    inv = 1.0 / (ROPE_BASE ** (np.arange(0, HD, 2, dtype=np.float32) / HD))
    t = np.arange(S, dtype=np.float32)
    freqs = np.outer(t, inv).astype(np.float32)
    emb = np.concatenate([freqs, freqs], axis=1)  # [S, HD]
    cos = np.cos(emb).astype(np.float32)
    sin = np.sin(emb).astype(np.float32)

    import ml_dtypes

    bf = ml_dtypes.bfloat16
    ii = np.arange(128)
    m0 = (ii[:, None] <= ii[None, :]).astype(bf)
    m4 = (ii[:, None] > ii[None, :]).astype(bf)
    m04 = np.ascontiguousarray(np.concatenate([m0, m4], axis=1))

    xTs, cos2s, sin2s = [], [], []
    for b in range(B):
        xTs.append(np.ascontiguousarray(hidden[b].T).astype(bf))
        cosT = np.ascontiguousarray(cos[pos[b]].T)  # [HD, S]
        sinT = np.ascontiguousarray(sin[pos[b]].T)
        sinS = np.concatenate([-sinT[:32], sinT[32:]], axis=0)
        cos2s.append(np.tile(cosT, (2, 1)).astype(bf))
        sin2s.append(np.tile(sinS, (2, 1)).astype(bf))

    in_maps = []
    for c in range(NCORES):
        b = c // 4
        h0 = NH * (c % 4)
        col = h0 * HD
        w_q = caw[:, col : col + NH * HD]
        w_k = caw[:, D + col : D + col + NH * HD]
        w_v = caw[:, 2 * D + col : 2 * D + col + NH * HD]
        wqkv = np.ascontiguousarray(
            np.concatenate([w_q, w_k, w_v], axis=1)
        ).astype(bf)
        b_q = cab[col : col + NH * HD]
        b_k = cab[D + col : D + col + NH * HD]
        bqk = np.ascontiguousarray(
            np.concatenate([b_q, b_k]).reshape(4, 128).T
        )  # [128, 4]: partition = col within tile
        wp = np.ascontiguousarray(cpw[col : col + NH * HD, :]).astype(bf)
        in_maps.append(
            {
                "xT": xTs[b],
                "wqkv": wqkv,
                "bqk": bqk,
                "wp": wp,
                "cos2": cos2s[b],
                "sin2": sin2s[b],
                "m04": m04,
            }
        )
    return in_maps


def _assemble(results, cab, cpw, cpb):
    """Host all-reduce of the 4 per-batch partials + biases.

    The V-bias contribution is position-independent after softmax
    (attn rows sum to 1), so it folds into a constant row vector:
    bias_v @ c_proj_w.
    """
    vrow = cab[2 * D :].astype(np.float32) @ cpw.astype(np.float32)
    bias = cpb.astype(np.float32) + vrow
    y = np.empty((B, S, D), dtype=np.float32)
    for b in range(B):
        acc = results[4 * b]["out"].astype(np.float32)
        for c in range(4 * b + 1, 4 * b + 4):
            acc = acc + results[c]["out"].astype(np.float32)
        y[b] = acc + bias[None, :]
    return y


def kernel(**inputs):
    from concourse import bass_utils

    hidden = np.asarray(inputs["hidden_states"], dtype=np.float32)
    pos = np.asarray(inputs["position_ids"]).astype(np.int64)
    caw = np.asarray(inputs["c_attn_w"], dtype=np.float32)
    cab = np.asarray(inputs["c_attn_b"], dtype=np.float32)
    cpw = np.asarray(inputs["c_proj_w"], dtype=np.float32)
    cpb = np.asarray(inputs["c_proj_b"], dtype=np.float32)

    in_maps = _host_inputs(hidden, pos, caw, cab, cpw)
    nc = _build_nc()
    res = bass_utils.run_bass_kernel_spmd(nc, in_maps, list(range(NCORES)))
    return _assemble(res.results, cab, cpw, cpb)


# revision 24
# speedup vs baseline: 1.9475x; 1.0445x over previous
"""Trainium2 Bass kernel: GPT2-style windowed attention (DecisionTransformer).

Full-input contract: kernel(**inputs) -> [B, S, D] float32.

Sharding: batch*heads across 8 cores (core c -> batch c//4, heads 4*(c%4)..+4).
Each core: column-sliced c_attn, full windowed attention for its 4 heads,
row-sliced c_proj producing a partial [S, D] output; host sums partials
(the "all-reduce") and adds c_proj bias + V-bias contribution once.

Layout / schedule choices:
  - all matmul operands are bf16 (tolerance is 2e-2; bf16 keeps PE at
    1 cyc/row and halves HBM traffic). PSUM accumulation stays fp32.
  - hidden is sent pre-transposed (xT [D, S]) so QK projections emit
    qT/kT directly in [head*dim, seq] layout; V is projected in
    [seq, head*dim] layout with a ones-column per head so attn@V
    accumulates softmax denominators in PSUM row 64 for free. V-bias
    folds into the host-side reduce (softmax rows sum to 1).
  - the whole kernel is software-pipelined: projections advance per
    512-col seq chunk, and attention q-quarters (scores+exp for all 4
    heads, kj-major attn@V, normalize, c_proj slice, output DMA) are
    issued as soon as their chunk dependencies are met. ACT does only
    exp during attention (the gating engine); evacuations go to DVE.
  - rope: rotate_half via 4 cross-quadrant 32-partition copies (2 DVE,
    2 GPSIMD) -- no DMA, no partition-swap latency chain.
  - scores for a head pair (partitions 0-63 / 64-127, K=64 each) are
    issued back-to-back so the PE runs them as concurrent row-tiles.
  - a dozen dummy matmuls at t=0 keep the PE busy (and the HAM
    clock-gate open) while the first input DMAs land.
"""

import sys

import numpy as np

sys.path.insert(0, "/opt/trn_rl_repo")

B, S, D = 2, 2048, 1024
H, HD = 16, 64
WINDOW = 512
ROPE_BASE = 4000.0
NCORES = 8
NH = 4          # heads per core
KT = D // 128   # 8 contraction tiles for c_attn
NB = S // 128   # 16 seq blocks
WB = WINDOW // 128  # 4 -> band spans up to 5 query blocks per key block


def _build_nc(debug_taps=False):
    import concourse.bass as bass
    from concourse import bacc, library_config, mybir
    import concourse.tile as tile

    f32 = mybir.dt.float32
    bf16 = mybir.dt.bfloat16
    Exp = mybir.ActivationFunctionType.Exp
    mult = mybir.AluOpType.mult
    ts = bass.ts
    ds = bass.ds

    nc = bacc.Bacc("TRN2")

    xT_d = nc.dram_tensor("xT", [D, S], bf16, kind="ExternalInput")
    wqkv_d = nc.dram_tensor("wqkv", [D, 3 * NH * HD], bf16, kind="ExternalInput")
    bqk_d = nc.dram_tensor("bqk", [128, 4], f32, kind="ExternalInput")
    wp_d = nc.dram_tensor("wp", [NH * HD, D], bf16, kind="ExternalInput")
    cos2_d = nc.dram_tensor("cos2", [128, S], bf16, kind="ExternalInput")
    sin2_d = nc.dram_tensor("sin2", [128, S], bf16, kind="ExternalInput")
    m04_d = nc.dram_tensor("m04", [128, 256], bf16, kind="ExternalInput")
    out_d = nc.dram_tensor("out", [S, D], bf16, kind="ExternalOutput")

    HS = S // 4  # q-quarter span: po is 1 PSUM bank
    QB = NB // 4  # 4 q-blocks per quarter

    with tile.TileContext(nc) as tc:
        nc.gpsimd.load_library(library_config.attn)

        with (
            tc.tile_pool(name="persist", bufs=1) as pp,
            tc.tile_pool(name="psj", bufs=2, space="PSUM") as psj_pool,
            tc.tile_pool(name="ps", bufs=2, space="PSUM") as ps_pool,
            tc.tile_pool(name="pso", bufs=2, space="PSUM") as pso_pool,
            tc.tile_pool(name="xw", bufs=1) as xw_pool,
            tc.tile_pool(name="ropetmp", bufs=4) as tmp_pool,
            tc.tile_pool(name="et", bufs=36) as e_pool,
            tc.tile_pool(name="rb", bufs=3) as rb_pool,
            tc.tile_pool(name="yo", bufs=3) as y_pool,
        ):
            # prewarm tile: zeros, matmul'd while input DMAs land
            zb = pp.tile([128, 512], bf16, tag="zb")
            nc.vector.memset(zb[:], 0.0)

            bqk_t = pp.tile([128, 4], f32, tag="bqk")
            nc.sync.dma_start(bqk_t[:], bqk_d[:])
            # m04[:, 0, :] = diag-block mask (kk<=qq); [:, 1, :] = window
            # edge mask (kk>qq) — applied as one two-region strided op
            m04t = pp.tile([128, 2, 128], bf16, tag="m04")
            nc.sync.dma_start(
                m04t[:], m04_d[:].rearrange("p (a b) -> p a b", a=2)
            )

            qk = [
                pp.tile([128, S], bf16, tag=f"qk{c}", name=f"qk{c}") for c in range(4)
            ]
            CV = NH * 65  # 260: per head 64 v-cols + 1 ones col
            vbig = pp.tile([128, NB, CV], bf16, tag="vbig")
            outH = pp.tile([128, 2, S], bf16, tag="outH")
            wpt = pp.tile([128, 2, D], bf16, tag="wpt")

            wbig = xw_pool.tile([128, KT, 3 * NH * HD], bf16, tag="wbig")
            xbig = xw_pool.tile([128, KT, S], bf16, tag="xbig")
            cos2 = xw_pool.tile([128, S], bf16, tag="cos2")
            sin2 = xw_pool.tile([128, S], bf16, tag="sin2")

            # DMA order = need order: v-cols, x first chunk, qk-cols, rope
            # tables, rest of x, c_proj weights.
            for kt in range(KT):
                nc.sync.dma_start(wbig[:, kt, :], wqkv_d[ts(kt, 128), :])
            for kt in range(KT):
                nc.sync.dma_start(xbig[:, kt, 0:512], xT_d[ts(kt, 128), 0:512])
            nc.sync.dma_start(cos2[:], cos2_d[:])
            nc.sync.dma_start(sin2[:], sin2_d[:])
            for kt in range(KT):
                nc.sync.dma_start(xbig[:, kt, 512:1024], xT_d[ts(kt, 128), 512:1024])
            for kt in range(KT):
                nc.sync.dma_start(
                    xbig[:, kt, 1024:2048], xT_d[ts(kt, 128), 1024:2048]
                )
            for k2 in range(2):
                nc.sync.dma_start(wpt[:, k2, :], wp_d[ts(k2, 128), :])

            # PE prewarm: keep the tensor engine busy (and the HAM
            # clock-gate open) while the first input DMAs land.
            for w in range(24):
                psw = psj_pool.tile([128, 512], f32, tag="psj", name="psw")
                nc.tensor.matmul(
                    psw[:], zb[:, 0:128], zb[:, 0:512],
                    start=True, stop=True,
                )

            # ---------------- building blocks ----------------
            eTs = [dict() for _ in range(NH)]  # [h][kj] -> masked exp'd scores

            def proj_v(sb):
                vsb = vbig[:, sb, :].rearrange("p (h c) -> p h c", c=65)
                nc.vector.memset(vsb[:, :, 64:65], 1.0)
                psv = psj_pool.tile([128, 256], f32, tag="psj", name="psv")
                for kt in range(KT):
                    nc.tensor.matmul(
                        psv[:],
                        xbig[:, kt, ts(sb, 128)],
                        wbig[:, kt, ds(2 * NH * HD, NH * HD)],
                        start=(kt == 0),
                        stop=(kt == KT - 1),
                    )
                nc.vector.tensor_copy(
                    vsb[:, :, 0:64],
                    psv[:].rearrange("p (h c) -> p h c", c=64),
                )

            def proj_qk(c, sc):
                psb = psj_pool.tile([128, 512], f32, tag="psj", name="psb")
                for kt in range(KT):
                    nc.tensor.matmul(
                        psb[:],
                        wbig[:, kt, ts(c, 128)],
                        xbig[:, kt, ts(sc, 512)],
                        start=(kt == 0),
                        stop=(kt == KT - 1),
                    )
                nc.scalar.add(
                    qk[c][:, ts(sc, 512)], psb[:], bqk_t[:, c : c + 1]
                )
                # rope: rotate_half via partition-swap SBUF DMAs on the
                # sync/gpsimd queues (sign is folded into the sin table)
                qc = qk[c][:, ts(sc, 512)]
                tmp = tmp_pool.tile([128, 512], bf16, tag="ropetmp", name="tmp")
                dma_engs = [nc.sync, nc.gpsimd, nc.sync, nc.gpsimd]
                for g in range(2):
                    b0 = g * 64
                    dma_engs[2 * g].dma_start(
                        tmp[b0 : b0 + 32, :],
                        qk[c][b0 + 32 : b0 + 64, ts(sc, 512)],
                    )
                    dma_engs[2 * g + 1].dma_start(
                        tmp[b0 + 32 : b0 + 64, :],
                        qk[c][b0 : b0 + 32, ts(sc, 512)],
                    )
                nc.vector.tensor_tensor(
                    tmp[:], tmp[:], sin2[:, ts(sc, 512)], op=mult
                )
                nc.vector.tensor_tensor(qc, qc, cos2[:, ts(sc, 512)], op=mult)
                nc.vector.tensor_add(qc, qc, tmp[:])

            def scores_exp(h, kj):
                # transposed scores sT[k, q] for the full band of kj
                # (5 q-blocks), exp'd on ACT, masked on DVE
                hb = (h % 2) * 64
                qt = qk[h // 2]
                kt_ = qk[2 + h // 2]
                nq = min(WB + 1, NB - kj)
                pss = ps_pool.tile([128, 640], f32, tag="ps", name="pss")
                n1 = min(512, nq * 128)
                n2 = nq * 128 - n1
                lhs_k = kt_[hb : hb + 64, ts(kj, 128)]
                nc.tensor.matmul(
                    pss[:, 0:n1],
                    lhs_k,
                    qt[hb : hb + 64, ds(kj * 128, n1)],
                    start=True,
                    stop=True,
                )
                if n2:
                    nc.tensor.matmul(
                        pss[:, 512 : 512 + n2],
                        lhs_k,
                        qt[hb : hb + 64, ds(kj * 128 + 512, n2)],
                        start=True,
                        stop=True,
                    )
                return pss, nq

            def exp_mask(h, kj, pss, nq):
                eT = e_pool.tile([128, 640], bf16, tag="et", name="eT")
                nc.scalar.activation(
                    eT[:, 0 : nq * 128], pss[:, 0 : nq * 128], Exp, scale=0.125
                )
                # banded mask: diag block keeps kk<=qq, window edge kk>qq.
                # Both 128-col regions are masked in one strided DVE op.
                if nq == WB + 1:
                    e2 = eT[:, 0:640].rearrange("p (x y) -> p x y", y=128)[
                        :, 0:5:4, :
                    ]
                    nc.vector.tensor_tensor(e2, e2, m04t[:], op=mult)
                else:
                    nc.vector.tensor_tensor(
                        eT[:, 0:128], eT[:, 0:128], m04t[:, 0, :], op=mult
                    )
                eTs[h][kj] = eT

            def evac_q(po, h, qtr):
                # normalize by denominators (PSUM row 64) into outH.
                # custom-DVE must not read PSUM: stage denom row via ACT.
                hb = (h % 2) * 64
                rb = rb_pool.tile([64, HS], f32, tag="rb", name="rb")
                nc.scalar.copy(rb[0:1, :], po[64:65, :])
                nc.vector.reciprocal_approx_fast(rb[0:1, :], rb[0:1, :])
                nc.gpsimd.partition_broadcast(rb[:], rb[0:1, :])
                nc.vector.tensor_tensor(
                    outH[hb : hb + 64, h // 2, qtr * HS : (qtr + 1) * HS],
                    po[0:64, :],
                    rb[:],
                    op=mult,
                )

            def attnv(h, qtr):
                # kj-major attn@V: each V block loads once and streams its
                # whole q-span (per-element has_written bits handle the
                # staggered accumulation regions)
                kjlo = max(0, 4 * qtr - WB)
                kjhi = 4 * qtr + QB - 1
                po = pso_pool.tile([65, HS], f32, tag="pso", name="po")
                for kj in range(kjlo, kjhi + 1):
                    qlo = max(4 * qtr, kj)
                    qhi = min(4 * qtr + QB - 1, kj + WB)
                    off = (qlo - kj) * 128
                    n = (qhi - qlo + 1) * 128
                    nc.tensor.matmul(
                        po[:, ds((qlo - 4 * qtr) * 128, n)],
                        vbig[:, kj, h * 65 : h * 65 + 65],
                        eTs[h][kj][:, off : off + n],
                        start=(kj == kjlo),
                        stop=(kj == kjhi),
                    )
                evac_q(po, h, qtr)

            def cproj(sb):
                # two 1-bank psum groups so c_proj never contends with the
                # scores pool
                yt = y_pool.tile([128, D], bf16, tag="yo", name="yt")
                for ncol in range(2):
                    psp = psj_pool.tile([128, 512], f32, tag="psj", name="psp")
                    for k2 in range(2):
                        nc.tensor.matmul(
                            psp[:],
                            outH[:, k2, ts(sb, 128)],
                            wpt[:, k2, ts(ncol, 512)],
                            start=(k2 == 0),
                            stop=(k2 == 1),
                        )
                    if (sb + ncol) % 2 == 0:
                        nc.scalar.copy(yt[:, ts(ncol, 512)], psp[:])
                    else:
                        nc.vector.tensor_copy(yt[:, ts(ncol, 512)], psp[:])
                nc.sync.dma_start(out_d[ts(sb, 128), :], yt[:])

            def scores_kj(kj):
                # scores+exp for one key block, pair-interleaved so the
                # two heads' K=64 matmuls run as concurrent PE row-tiles
                for pair in range(2):
                    s0 = scores_exp(2 * pair, kj)
                    s1 = scores_exp(2 * pair + 1, kj)
                    exp_mask(2 * pair, kj, *s0)
                    exp_mask(2 * pair + 1, kj, *s1)

            def wave(qtr):
                # Scores run a full wave ahead: this wave consumes quarter
                # qtr's eT tiles (exp'd during wave qtr-1) and produces
                # quarter qtr+1's. Projection chunks lead the wave so the
                # PE stays dense (and the clock-gate warm) while ACT drains
                # the exp backlog; attn@V heads, new score blocks, and the
                # previous quarter's c_proj interleave so no engine queue
                # builds a deep backlog in front of a dependency.
                ch = qtr + 2
                if ch < 4:
                    for j in range(4):
                        proj_v(4 * ch + j)
                        proj_qk((0, 2, 1, 3)[j], ch)
                for i in range(4):
                    if qtr >= 1:
                        cproj(4 * (qtr - 1) + i)
                    attnv(i, qtr)
                    if qtr < 3:
                        scores_kj(4 * qtr + 4 + i)

            # ---------------- pipelined schedule ----------------
            for sc in range(2):
                for sb in range(4 * sc, 4 * sc + 4):
                    proj_v(sb)
                for c in (0, 2, 1, 3):
                    proj_qk(c, sc)
            for kj in range(4):
                scores_kj(kj)
            for qtr in range(4):
                wave(qtr)
            for sb in range(12, 16):
                cproj(sb)

    nc.compile()
    return nc


def _host_inputs(hidden, pos, caw, cab, cpw):
    """Build the 8 per-core input maps."""
    inv = 1.0 / (ROPE_BASE ** (np.arange(0, HD, 2, dtype=np.float32) / HD))
    t = np.arange(S, dtype=np.float32)
    freqs = np.outer(t, inv).astype(np.float32)
    emb = np.concatenate([freqs, freqs], axis=1)  # [S, HD]
    cos = np.cos(emb).astype(np.float32)
    sin = np.sin(emb).astype(np.float32)

    import ml_dtypes

    bf = ml_dtypes.bfloat16
    ii = np.arange(128)
    m0 = (ii[:, None] <= ii[None, :]).astype(bf)
    m4 = (ii[:, None] > ii[None, :]).astype(bf)
    m04 = np.ascontiguousarray(np.concatenate([m0, m4], axis=1))

    xTs, cos2s, sin2s = [], [], []
    for b in range(B):
        xTs.append(np.ascontiguousarray(hidden[b].T).astype(bf))
        cosT = np.ascontiguousarray(cos[pos[b]].T)  # [HD, S]
        sinT = np.ascontiguousarray(sin[pos[b]].T)
        sinS = np.concatenate([-sinT[:32], sinT[32:]], axis=0)
        cos2s.append(np.tile(cosT, (2, 1)).astype(bf))
        sin2s.append(np.tile(sinS, (2, 1)).astype(bf))

    in_maps = []
    for c in range(NCORES):
        b = c // 4
        h0 = NH * (c % 4)
        col = h0 * HD
        w_q = caw[:, col : col + NH * HD]
        w_k = caw[:, D + col : D + col + NH * HD]
        w_v = caw[:, 2 * D + col : 2 * D + col + NH * HD]
        wqkv = np.ascontiguousarray(
            np.concatenate([w_q, w_k, w_v], axis=1)
        ).astype(bf)
        b_q = cab[col : col + NH * HD]
        b_k = cab[D + col : D + col + NH * HD]
        bqk = np.ascontiguousarray(
            np.concatenate([b_q, b_k]).reshape(4, 128).T
        )  # [128, 4]: partition = col within tile
        wp = np.ascontiguousarray(cpw[col : col + NH * HD, :]).astype(bf)
        in_maps.append(
            {
                "xT": xTs[b],
                "wqkv": wqkv,
                "bqk": bqk,
                "wp": wp,
                "cos2": cos2s[b],
                "sin2": sin2s[b],
                "m04": m04,
            }
        )
    return in_maps


def _assemble(results, cab, cpw, cpb):
    """Host all-reduce of the 4 per-batch partials + biases.

    The V-bias contribution is position-independent after softmax
    (attn rows sum to 1), so it folds into a constant row vector:
    bias_v @ c_proj_w.
    """
    vrow = cab[2 * D :].astype(np.float32) @ cpw.astype(np.float32)
    bias = cpb.astype(np.float32) + vrow
    y = np.empty((B, S, D), dtype=np.float32)
    for b in range(B):
        acc = results[4 * b]["out"].astype(np.float32)
        for c in range(4 * b + 1, 4 * b + 4):
            acc = acc + results[c]["out"].astype(np.float32)
        y[b] = acc + bias[None, :]
    return y


def kernel(**inputs):
    from concourse import bass_utils

    hidden = np.asarray(inputs["hidden_states"], dtype=np.float32)
    pos = np.asarray(inputs["position_ids"]).astype(np.int64)
    caw = np.asarray(inputs["c_attn_w"], dtype=np.float32)
    cab = np.asarray(inputs["c_attn_b"], dtype=np.float32)
    cpw = np.asarray(inputs["c_proj_w"], dtype=np.float32)
    cpb = np.asarray(inputs["c_proj_b"], dtype=np.float32)

    in_maps = _host_inputs(hidden, pos, caw, cab, cpw)
    nc = _build_nc()
    res = bass_utils.run_bass_kernel_spmd(nc, in_maps, list(range(NCORES)))
    return _assemble(res.results, cab, cpw, cpb)
